# revision 9
# baseline (speedup 1.0000x reference)
"""Trainium2 Bass kernel for nn_DTAM (differential transposed-attention module).

Sharding: 8 cores = batch(4) x head(2). Each core computes its (b, h) shard
end-to-end; host does LayerNorm scale precompute, weight folding, and the
final partial-sum + residual merge.

v3 design (vs v2 baseline at 571us):
- The pointwise C->2C + depthwise 3x3 convs stay fused into a dense 3x3
  (9 tap-matmuls, fp8 DoubleRow), but each tap's weight is now loaded ONCE
  per 28-row rotation via an explicit ldweights and streamed over 7
  non-self-loading matmuls of 512 px each (ins.ldweights=False). The v2
  kernel paid a ~122ns LDWEIGHTS for every 256-px matmul (LDW-port bound);
  v3 amortizes it 7x while doubling the moving window.
- Channel-attention scores accumulate directly in a persistent PSUM tile
  (1 bank) across all 128 row-block matmuls; conv rotations use the other
  7 banks.
- Phase C out-projection runs in fp8 DoubleRow (both 96-ch halves
  contracted in one matmul), with the pow2 weight scale compensated
  exactly inside the rsqrt activation's input scale/bias.
- Software pipelining: score matmuls for rotation r are emitted after the
  conv of rotation r+1, so their DMA transposes overlap conv streaming.
"""

import numpy as np
import ml_dtypes
from contextlib import ExitStack

BF16 = ml_dtypes.bfloat16
FP8 = ml_dtypes.float8_e4m3fn

# ---- problem constants (hardcoded per contest rules) ----
B, C, H, W = 4, 192, 128, 128
HEADS = 2
N = H * W                 # 16384
HC = 96                   # half-channels per head (q1/q2 split)
LAM_INIT = 0.8
CH = 512                  # phase C chunk

# conv rotations: (start_row, n_windows, chunk_split_in_windows)
ROTS = [(0, 7, 4), (28, 7, 4), (56, 7, 4), (84, 7, 4), (112, 4, 2)]

_CACHED = {}


def _build_program():
    import concourse.bass as bass
    import concourse.bacc as bacc
    import concourse.tile as tile
    from concourse import mybir

    f32 = mybir.dt.float32
    bf16 = mybir.dt.bfloat16
    fp8 = mybir.dt.float8e4
    AF = mybir.ActivationFunctionType
    OP = mybir.AluOpType
    AX = mybir.AxisListType
    DR = mybir.MatmulPerfMode.DoubleRow

    nc = bacc.Bacc("TRN2", target_bir_lowering=False, debug=False,
                   num_devices=8)

    # ---- DRAM I/O ----
    # xs fp8, channel halves interleaved, rows 0/129 zero-padded:
    # xs8[c, j, 1+y, x] = xs[c + 96*j, y, x]
    XJ = (H + 2) * W
    xs_d = nc.dram_tensor("xs8", [96, 2 * XJ], fp8, kind="ExternalInput")
    # fused conv weights per tensor/half: [96(c), 9(t), 2(j), 96(o)]
    w_d = {}
    for p in ("q", "k", "v"):
        for hf in range(2):
            w_d[(p, hf)] = nc.dram_tensor(
                f"w{p}{hf}", [96, 9 * 2 * 96], fp8, kind="ExternalInput")
    # fp8 out-projection weights: wo8[mt][c, j, o]
    wo8_d = [nc.dram_tensor(f"wo8_{mt}", [96, 2 * 96], fp8,
                            kind="ExternalInput") for mt in range(2)]
    ones96_d = nc.dram_tensor("ones96", [96, 1], bf16, kind="ExternalInput")
    ones1_d = nc.dram_tensor("ones1", [1, 128], bf16, kind="ExternalInput")
    ident_d = nc.dram_tensor("ident", [96, 96], bf16, kind="ExternalInput")
    neglam_d = nc.dram_tensor("neglam", [128, 1], f32, kind="ExternalInput")
    tsc_d = nc.dram_tensor("tsc", [96, 2], f32, kind="ExternalInput")
    epsd_d = nc.dram_tensor("epsd", [1, 1], f32, kind="ExternalInput")
    out_d = nc.dram_tensor("out", [192, N], f32, kind="ExternalOutput")

    # tap t in 0..8 -> spatial offset (oy, ox), correlation convention
    OFFS = [(t // 3 - 1, t % 3 - 1) for t in range(9)]
    TAP_ORDER = [4] + [t for t in range(9) if t != 4]

    def xr(ox):
        if ox == -1:
            return (1, 128), (0, 127)
        if ox == 1:
            return (0, 127), (1, 128)
        return (0, 128), (0, 128)

    with tile.TileContext(nc) as tc, ExitStack() as ctx:
        cst = ctx.enter_context(tc.tile_pool(name="cst", bufs=1))
        res = ctx.enter_context(tc.tile_pool(name="res", bufs=1))

        # ---- load constants ----
        xs8 = cst.tile([96, 2, H + 2, W], fp8, name="xs8", tag="xs8")
        nc.sync.dma_start(xs8[:].rearrange("p a b c -> p (a b c)"), xs_d[:])
        wt = {}
        for p in ("q", "k", "v"):
            for hf in range(2):
                t = cst.tile([96, 9, 2, 96], fp8, name=f"w{p}{hf}",
                             tag=f"w{p}{hf}")
                nc.sync.dma_start(t[:].rearrange("p a b c -> p (a b c)"),
                                  w_d[(p, hf)][:])
                wt[(p, hf)] = t
        wo8 = []
        for mt in range(2):
            t = cst.tile([96, 2, 96], fp8, name=f"wo8{mt}", tag=f"wo8{mt}")
            nc.sync.dma_start(t[:].rearrange("p a b -> p (a b)"),
                              wo8_d[mt][:])
            wo8.append(t)
        ones96 = cst.tile([96, 1], bf16, name="o96", tag="o96")
        ones1 = cst.tile([1, 128], bf16, name="o1", tag="o1")
        ident = cst.tile([96, 96], bf16, name="id", tag="id")
        neglam = cst.tile([128, 1], f32, name="nl", tag="nl")
        tsc = cst.tile([96, 2], f32, name="tsc", tag="tsc")
        epsd = cst.tile([1, 1], f32, name="epsd", tag="epsd")
        nc.sync.dma_start(ones96[:], ones96_d[:])
        nc.sync.dma_start(ones1[:], ones1_d[:])
        nc.sync.dma_start(ident[:], ident_d[:])
        nc.sync.dma_start(neglam[:], neglam_d[:])
        nc.sync.dma_start(tsc[:], tsc_d[:])
        nc.sync.dma_start(epsd[:], epsd_d[:])

        # resident dwv halves (bf16)
        dwv_res = [res.tile([96, N], bf16, name=f"dwv{i}", tag=f"dwv{i}")
                   for i in range(2)]

        smx = ctx.enter_context(tc.tile_pool(name="smx", bufs=1))

        # ================= PHASE A =================
        GROUPS = [("q", 0), ("q", 1), ("k", 0), ("k", 1), ("v", 0), ("v", 1)]
        n_blk_total = H  # one 128-px score block per image row, per hf

        pa_stack = ExitStack()
        stg = pa_stack.enter_context(tc.tile_pool(name="stg", bufs=2))
        tro = pa_stack.enter_context(tc.tile_pool(name="tro", bufs=1))
        cvps = pa_stack.enter_context(
            tc.tile_pool(name="cvps", bufs=1, space="PSUM"))
        scps = pa_stack.enter_context(
            tc.tile_pool(name="scps", bufs=1, space="PSUM"))

        # persistent score accumulator: [96, hf, 96] in one PSUM bank
        psc = scps.tile([96, 2, 96], f32, name="psc", tag="psc")
        blk_count = [0, 0]  # per-hf emitted score blocks

        def emit_scores(trts, nrows):
            # trts: {(p,hf): tile [128, nrows, 96]}
            for hf in range(2):
                for blk in range(nrows):
                    nc.tensor.matmul(
                        psc[:, hf, :],
                        trts[("q", hf)][:, blk, :],
                        trts[("k", hf)][:, blk, :],
                        start=(blk_count[hf] == 0),
                        stop=(blk_count[hf] == n_blk_total - 1),
                        skip_group_check=True)
                    blk_count[hf] += 1

        pending_scores = []  # list of (trts, nrows) awaiting emission

        vcopy_rr = [0]

        for (r0, nw, cw0) in ROTS:
            trts = {}
            stgs = {}
            for (p, hf) in GROUPS:
                # psum window tiles for this group (reuse 7 tags -> 7 banks)
                pst = [cvps.tile([96, 4, 128], f32, name=f"cv{w}",
                                 tag=f"cv{w}") for w in range(nw)]
                for ti, t in enumerate(TAP_ORDER):
                    oy, ox = OFFS[t]
                    (a0, a1), (b0, b1) = xr(ox)
                    nc.tensor.ldweights(wt[(p, hf)][:, t, :, :],
                                        perf_mode=DR)
                    for w in range(nw):
                        rw = r0 + 4 * w
                        mm = nc.tensor.matmul(
                            pst[w][:, :, a0:a1],
                            wt[(p, hf)][:, t, :, :],
                            xs8[:, :, 1 + rw + oy:5 + rw + oy, b0:b1],
                            start=(ti == 0), stop=(ti == 8),
                            perf_mode=DR,
                            skip_group_check=True)
                        mm.ins.ldweights = False
                # evacuate
                if p == "v":
                    for w in range(nw):
                        seg = (r0 + 4 * w) * W
                        dst = dwv_res[hf][:, seg:seg + 512]\
                            .rearrange("p (r x) -> p r x", x=128)
                        # alternate v evac between ACT and DVE
                        # (GPSIMD cannot read PSUM)
                        rr = vcopy_rr[0] % 2
                        vcopy_rr[0] += 1
                        if rr == 0:
                            nc.scalar.copy(dst, pst[w][:])
                        else:
                            nc.vector.tensor_copy(dst, pst[w][:])
                else:
                    for ci, (wlo, whi) in enumerate(
                            ((0, cw0), (cw0, nw))):
                        ncw = whi - wlo
                        st = stg.tile([96, 4 * ncw, 128], bf16,
                                      name=f"s{p}{hf}{ci}",
                                      tag=f"s{p}{hf}{ci}")
                        stgs[(p, hf, ci)] = st
                        for w in range(wlo, whi):
                            dst = st[:, 4 * (w - wlo):4 * (w - wlo) + 4, :]
                            if p == "q":
                                nc.scalar.copy(dst, pst[w][:])
                            else:
                                nc.vector.tensor_copy(dst, pst[w][:])
            # DMA transposes for this rotation's q/k chunks
            for ci, ncw in ((0, cw0), (1, nw - cw0)):
                tr = {}
                for (p, hf) in GROUPS[:4]:
                    tt = tro.tile([128, 4 * ncw, 96], bf16,
                                  name=f"t{p}{hf}{ci}", tag=f"t{p}{hf}{ci}")
                    nc.sync.dma_start_transpose(
                        tt[:], stgs[(p, hf, ci)][:].rearrange(
                            "p r x -> p (r x)"))
                    tr[(p, hf)] = tt
                pending_scores.append((tr, 4 * ncw))
            # emit score matmuls for the PREVIOUS rotation (pipelined)
            while len(pending_scores) > 2:
                emit_scores(*pending_scores.pop(0))
        while pending_scores:
            emit_scores(*pending_scores.pop(0))

        # ================= PHASE B: softmax + attn =================
        ex = []
        rr_ = []
        for hf in range(2):
            scl = smx.tile([96, 96], f32, name=f"scl{hf}", tag=f"scl{hf}")
            nc.vector.tensor_scalar(scl[:], psc[:, hf, :], tsc[:, hf:hf + 1],
                                    None, OP.mult)
            nm = smx.tile([96, 1], f32, name=f"nm{hf}", tag=f"nm{hf}")
            nc.vector.tensor_reduce(nm[:], scl[:], AX.X, OP.max, negate=True)
            e = smx.tile([96, 96], f32, name=f"e{hf}", tag=f"e{hf}")
            nc.scalar.activation(e[:], scl[:], AF.Exp, bias=nm[:, 0:1])
            sm = smx.tile([96, 1], f32, name=f"sm{hf}", tag=f"sm{hf}")
            nc.vector.tensor_reduce(sm[:], e[:], AX.X, OP.add)
            r = smx.tile([96, 1], f32, name=f"r{hf}", tag=f"r{hf}")
            nc.vector.reciprocal(r[:], sm[:])
            ex.append(e)
            rr_.append(r)
        pa_stack.close()

        atstack = ExitStack()
        atps = atstack.enter_context(
            tc.tile_pool(name="atps", bufs=1, space="PSUM"))
        r2n = smx.tile([96, 1], f32, name="r2n", tag="r2n")
        nc.vector.tensor_scalar(r2n[:], rr_[1][:], neglam[0:96, 0:1],
                                None, OP.mult)
        a1 = smx.tile([96, 96], f32, name="a1", tag="a1")
        nc.scalar.mul(a1[:], ex[0][:], rr_[0][:, 0:1])
        attn = smx.tile([96, 96], bf16, name="attn", tag="attn")
        nc.vector.scalar_tensor_tensor(attn[:], ex[1][:], r2n[:, 0:1],
                                       a1[:], OP.mult, OP.add)
        pt = atps.tile([96, 96], bf16, name="pt", tag="pt")
        nc.tensor.transpose(pt[:], attn[:], ident[:])
        attnT = smx.tile([96, 96], bf16, name="attnT", tag="attnT")
        nc.scalar.copy(attnT[:], pt[:])
        atstack.close()

        # ================= PHASE C =================
        with tc.tile_pool(name="yp", bufs=2) as yp, \
             tc.tile_pool(name="op_", bufs=2) as op_, \
             tc.tile_pool(name="yps", bufs=2, space="PSUM") as yps, \
             tc.tile_pool(name="sqps", bufs=1, space="PSUM") as sqps, \
             tc.tile_pool(name="rbps", bufs=1, space="PSUM") as rbps, \
             tc.tile_pool(name="ops", bufs=2, space="PSUM") as ops:
            for cc in range(N // CH):
                seg = cc * CH
                nc.tensor.ldweights(attnT[:])
                pys = []
                for hf in range(2):
                    py = yps.tile([96, CH], f32, name=f"y{hf}", tag=f"y{hf}")
                    mm = nc.tensor.matmul(py[:], attnT[:],
                                          dwv_res[hf][:, seg:seg + CH],
                                          start=True, stop=True,
                                          skip_group_check=True)
                    mm.ins.ldweights = False
                    pys.append(py)
                ys8 = yp.tile([96, 2, CH], fp8, name="ys8", tag="ys8")
                ysb = []
                yyb = []
                for hf in range(2):
                    ys = yp.tile([96, CH], bf16, name=f"ys{hf}",
                                 tag=f"ys{hf}")
                    if hf == 0:
                        nc.scalar.copy(ys[:], pys[hf][:])
                    else:
                        nc.vector.tensor_copy(ys[:], pys[hf][:])
                    yy = yp.tile([96, CH], bf16, name=f"yy{hf}",
                                 tag=f"yy{hf}")
                    nc.gpsimd.tensor_tensor(yy[:], ys[:], ys[:], OP.mult)
                    ysb.append(ys)
                    yyb.append(yy)
                # fp8 copies (scale 0.5 to stay in e4m3 range)
                nc.gpsimd.tensor_scalar(ys8[:, 0, :], ysb[0][:],
                                        0.5, None, OP.mult)
                nc.vector.tensor_scalar(ys8[:, 1, :], pys[1][:],
                                        0.5, None, OP.mult)
                pss = sqps.tile([1, CH], f32, name="ss", tag="ss")
                nc.tensor.matmul(pss[:], ones96[:], yyb[0][:],
                                 start=True, stop=False)
                nc.tensor.matmul(pss[:], ones96[:], yyb[1][:],
                                 start=False, stop=True)
                rsb = op_.tile([1, CH], bf16, name="rs", tag="rs")
                # rsb = (1/s_w) * rsqrt(mean(y^2) + eps'):
                #   scale = s_w^2/192 (pow2-exact), bias = eps' * s_w^2
                nc.scalar.activation(rsb[:], pss[:],
                                     AF.Abs_reciprocal_sqrt,
                                     bias=epsd[0:1, 0:1],
                                     scale=1.0 / 192.0)
                prb = rbps.tile([128, CH], f32, name="rb", tag="rb")
                nc.tensor.matmul(prb[:], ones1[:], rsb[:],
                                 start=True, stop=True)
                rbsb = op_.tile([128, CH], f32, name="rbs", tag="rbs")
                nc.vector.tensor_copy(rbsb[:], prb[:])
                for mt in range(2):
                    po = ops.tile([96, CH], f32, name="po", tag="po")
                    nc.tensor.matmul(po[:], wo8[mt][:], ys8[:],
                                     start=True, stop=True,
                                     perf_mode=DR,
                                     skip_group_check=True)
                    osb = op_.tile([96, CH], f32, name=f"os{mt}",
                                   tag=f"os{mt}")
                    nc.vector.tensor_tensor(osb[:], po[:],
                                            rbsb[0:96, :], OP.mult)
                    nc.sync.dma_start(
                        out_d[mt * 96:(mt + 1) * 96, seg:seg + CH],
                        osb[:])
    nc.compile()
    return nc


def _pow2_scale(maxabs, target=128.0):
    if maxabs <= 0:
        return 1.0
    return float(2.0 ** np.floor(np.log2(target / maxabs)))


def _prep_inputs(inputs):
    x = np.asarray(inputs["x"], np.float32)
    norm_w = np.asarray(inputs["norm_w"], np.float32)
    Wq = np.asarray(inputs["Wq"], np.float32)
    Wk = np.asarray(inputs["Wk"], np.float32)
    Wv = np.asarray(inputs["Wv"], np.float32)
    Dq = np.asarray(inputs["Dq"], np.float32)
    Dk = np.asarray(inputs["Dk"], np.float32)
    Dv = np.asarray(inputs["Dv"], np.float32)
    t1 = np.asarray(inputs["t1"], np.float32)
    t2 = np.asarray(inputs["t2"], np.float32)
    hn_w = np.asarray(inputs["hn_w"], np.float32)
    Wo = np.asarray(inputs["Wo"], np.float32)
    lam = float(np.exp(np.sum(inputs["lq1"] * inputs["lk1"],
                              dtype=np.float64))
                - np.exp(np.sum(inputs["lq2"] * inputs["lk2"],
                                dtype=np.float64))
                + LAM_INIT)

    # LayerNorm scale on host
    var = x.var(axis=1)                       # [B, H, W]
    s = 1.0 / np.sqrt(var + 1e-5)
    xs = (x * s[:, None, :, :]).reshape(B, C, H, W)

    Wq_f = Wq * norm_w[None, :]
    Wk_f = Wk * norm_w[None, :]
    Wv_f = Wv * norm_w[None, :]

    in_maps = []
    for core in range(8):
        b, h = core // 2, core % 2
        sl = slice(h * 192, (h + 1) * 192)
        m = {}
        # xs fp8 interleaved with zero-padded rows
        xpad = np.zeros((96, 2, H + 2, W), np.float32)
        xc = xs[b]                            # [192, H, W]
        xpad[:, 0, 1:H + 1, :] = xc[0:96]
        xpad[:, 1, 1:H + 1, :] = xc[96:192]
        m["xs8"] = np.clip(xpad, -224, 224).astype(FP8).reshape(96, -1)

        scales = {}
        for nm, Wf, Dd in (("q", Wq_f, Dq), ("k", Wk_f, Dk),
                           ("v", Wv_f, Dv)):
            Wh = Wf[sl]                       # [192 out, 192 in]
            dh = Dd[sl, 0].reshape(192, 9)    # [192 out, 9 taps]
            # K3[o, c, t] = Wh[o, c] * dh[o, t]
            K3 = Wh[:, :, None] * dh[:, None, :]
            sp = _pow2_scale(np.abs(K3).max())
            scales[nm] = sp
            K3s = np.clip(K3 * sp, -224, 224)
            for hf in range(2):
                # w8[c, t, j, o] = K3s[hf*96+o, c+96j, t]
                blk = K3s[hf * 96:(hf + 1) * 96]   # [96 o, 192 cg, 9 t]
                w8 = blk.transpose(1, 2, 0).reshape(2, 96, 9, 96)
                # [192 cg, 9 t, 96 o] -> split cg=(j, c): [2 j, 96 c, 9 t, 96 o]
                m[f"w{nm}{hf}"] = np.ascontiguousarray(
                    w8.transpose(1, 2, 0, 3)   # [96 c, 9 t, 2 j, 96 o]
                ).astype(FP8).reshape(96, -1)

        th = np.array([t1[h, 0, 0], t2[h, 0, 0]], np.float32)
        m["tsc"] = np.broadcast_to(
            (th / (scales["q"] * scales["k"]))[None, :], (96, 2)
        ).astype(np.float32).copy()

        # out-projection: lhsT[y-ch, out-ch] with hn_w * (1-lam) folded.
        Wo_hf = Wo[:, sl] * (hn_w[h] * (1.0 - LAM_INIT))[None, :]
        lhsT = Wo_hf.T.astype(np.float32)     # [192 y-ch, 192 out]
        # fp8: wo8[mt][c, j, o] = clip(lhsT[j*96+c, mt*96+o] * 2 * s_w)
        s_w = _pow2_scale(np.abs(lhsT).max() * 2.0)
        lw = np.clip(lhsT * (2.0 * s_w), -224, 224)
        lw = lw.reshape(2, 96, 2, 96)         # [j, c, mt, o]
        for mt in range(2):
            m[f"wo8_{mt}"] = np.ascontiguousarray(
                lw[:, :, mt, :].transpose(1, 0, 2)   # [c, j, o]
            ).astype(FP8).reshape(96, -1)
        # rsqrt compensation: pss = s_w^2 * sum(y^2) via ones96 = s_w^2,
        # bias = eps' * s_w^2 where eps' = 1e-6 * sv^2; both pow2-exact.
        m["epsd"] = np.full((1, 1),
                            1e-6 * scales["v"] ** 2 * s_w ** 2, np.float32)
        m["ones96"] = np.full((96, 1), s_w ** 2, BF16)
        m["ones1"] = np.ones((1, 128), BF16)
        m["ident"] = np.eye(96, dtype=BF16)
        m["neglam"] = np.full((128, 1), -lam, np.float32)
        in_maps.append(m)
    return in_maps


def kernel(**inputs):
    from concourse import bass_utils

    if "nc" not in _CACHED:
        _CACHED["nc"] = _build_program()
    nc = _CACHED["nc"]

    in_maps = _prep_inputs(inputs)
    results = bass_utils.run_bass_kernel_spmd(
        nc, in_maps, core_ids=list(range(8))).results

    x = np.asarray(inputs["x"], np.float32)
    out = np.empty((B, C, N), np.float32)
    for b in range(B):
        out[b] = results[2 * b]["out"] + results[2 * b + 1]["out"]
    out = out.reshape(B, C, H, W) + x
    return out.astype(np.float32)


# revision 12
# speedup vs baseline: 1.3283x; 1.3283x over previous
"""Trainium2 Bass kernel for nn_DTAM (differential transposed-attention module).

Sharding: 8 cores = batch(4) x head(2). Each core computes its (b, h) shard
end-to-end; host does LayerNorm scale precompute, weight folding, and the
final partial-sum + residual merge (including per-core fp8 descale).

v4 design (vs 571us v2 baseline):
- Dense fused 3x3 conv (fp8 DoubleRow) with M=128 OUTPUT PACKING: the 576
  output channels (q/k/v x 2 halves x 96) stream as 5 M-groups
  (4x128 + 1x64) instead of 6x96. The moving-operand port (2B/part/cyc)
  is the hard wall, so fewer streams = directly less PE time.
  Evacuation uses partition-shifted engine copies (PSUM part p -> SBUF
  part q, p != q), verified on HW.
- 4-row conv windows: each matmul streams 512 px (1016-elem moving AP),
  7 PSUM banks rotate per 28 rows; the 8th bank holds the persistent
  score accumulator psc (one accumulation chain per hf over all 128
  row-blocks).
- PE warm-up dummy matmuls run while the 3.2MB xs8 DMA lands, keeping the
  HAM clock-gate at full rate for the first conv rotation.
- Phase C is software-pipelined 5 stages deep across 512-px chunks, so
  the in-chunk PE->ACT->PE->...->DVE dependency chain never stalls PE:
    A: y matmuls (PE), ysb=cast (ACT)
    B: yy=ysb^2 (GPSIMD)
    C: stats matmul (PE), rsqrt (ACT)
    D: broadcast matmul (PE), rbsb copy (ACT), ys8=ysb*r fp8 (DVE)
    E: out-proj fp8-DoubleRow (PE), osb cast (DVE), DMA out
  RMS-norm r is folded into the fp8 proj input (|16*yhat| <= 222 < 448,
  mathematically bounded, no clipping); all scales are pow2-exact, and
  the final pow2 descale happens on host during the partial-sum merge.
"""

import numpy as np
import ml_dtypes
from contextlib import ExitStack

BF16 = ml_dtypes.bfloat16
FP8 = ml_dtypes.float8_e4m3fn

# ---- problem constants (hardcoded per contest rules) ----
B, C, H, W = 4, 192, 128, 128
HEADS = 2
N = H * W                 # 16384
HC = 96
LAM_INIT = 0.8
CH = 512                  # phase C chunk (px)
NCH = N // CH             # 32 chunks

# conv rotations: (start_row, n_windows, windows_in_first_transpose_chunk)
ROTS = [(0, 7, 4), (28, 7, 4), (56, 7, 4), (84, 7, 4), (112, 4, 2)]

# M-group packing of the 576 output channels, order q0,q1,k0,k1,v0,v1.
# Group g covers global channels [128g, 128g+Mg); segment list maps psum
# partition ranges to (tensor, hf, dest channel offset).
GROUP_M = [128, 128, 128, 128, 64]
GROUP_SEGS = [
    [(("q", 0), 0, 96, 0), (("q", 1), 96, 128, 0)],
    [(("q", 1), 0, 64, 32), (("k", 0), 64, 128, 0)],
    [(("k", 0), 0, 32, 64), (("k", 1), 32, 128, 0)],
    [(("v", 0), 0, 96, 0), (("v", 1), 96, 128, 0)],
    [(("v", 1), 0, 64, 32)],
]

_CACHED = {}


def _build_program():
    import concourse.bass as bass
    import concourse.bacc as bacc
    import concourse.tile as tile
    from concourse import mybir

    f32 = mybir.dt.float32
    bf16 = mybir.dt.bfloat16
    fp8 = mybir.dt.float8e4
    AF = mybir.ActivationFunctionType
    OP = mybir.AluOpType
    AX = mybir.AxisListType
    DR = mybir.MatmulPerfMode.DoubleRow

    nc = bacc.Bacc("TRN2", target_bir_lowering=False, debug=False,
                   num_devices=8)

    # ---- DRAM I/O ----
    XJ = (H + 2) * W
    xs_d = nc.dram_tensor("xs8", [96, 2 * XJ], fp8, kind="ExternalInput")
    wg_d = [nc.dram_tensor(f"wg{g}", [96, 9 * 2 * GROUP_M[g]], fp8,
                           kind="ExternalInput") for g in range(5)]
    wo8_d = [nc.dram_tensor(f"wo8_{mt}", [96, 2 * 96], fp8,
                            kind="ExternalInput") for mt in range(2)]
    w96_d = nc.dram_tensor("w96", [96, 1], bf16, kind="ExternalInput")
    ones1_d = nc.dram_tensor("ones1", [1, 96], bf16, kind="ExternalInput")
    ident_d = nc.dram_tensor("ident", [96, 96], bf16, kind="ExternalInput")
    neglam_d = nc.dram_tensor("neglam", [128, 1], f32, kind="ExternalInput")
    tsc_d = nc.dram_tensor("tsc", [96, 2], f32, kind="ExternalInput")
    epsd_d = nc.dram_tensor("epsd", [1, 1], f32, kind="ExternalInput")
    out_d = nc.dram_tensor("out", [192, N], f32, kind="ExternalOutput")

    OFFS = [(t // 3 - 1, t % 3 - 1) for t in range(9)]
    TAP_ORDER = [4] + [t for t in range(9) if t != 4]

    def xr(ox):
        if ox == -1:
            return (1, 128), (0, 127)
        if ox == 1:
            return (0, 127), (1, 128)
        return (0, 128), (0, 128)

    with tile.TileContext(nc) as tc, ExitStack() as ctx:
        cst = ctx.enter_context(tc.tile_pool(name="cst", bufs=1))
        res = ctx.enter_context(tc.tile_pool(name="res", bufs=1))

        # ---- constants: small DMAs first so warm-up can start early ----
        wt = []
        for g in range(5):
            t = cst.tile([96, 9, 2, GROUP_M[g]], fp8, name=f"wg{g}",
                         tag=f"wg{g}")
            nc.sync.dma_start(t[:].rearrange("p a b c -> p (a b c)"),
                              wg_d[g][:])
            wt.append(t)
        wo8 = []
        for mt in range(2):
            t = cst.tile([96, 2, 96], fp8, name=f"wo8{mt}", tag=f"wo8{mt}")
            nc.sync.dma_start(t[:].rearrange("p a b -> p (a b)"),
                              wo8_d[mt][:])
            wo8.append(t)
        w96 = cst.tile([96, 1], bf16, name="w96", tag="w96")
        ones1 = cst.tile([1, 96], bf16, name="o1", tag="o1")
        ident = cst.tile([96, 96], bf16, name="id", tag="id")
        neglam = cst.tile([128, 1], f32, name="nl", tag="nl")
        tsc = cst.tile([96, 2], f32, name="tsc", tag="tsc")
        epsd = cst.tile([1, 1], f32, name="epsd", tag="epsd")
        nc.sync.dma_start(w96[:], w96_d[:])
        nc.sync.dma_start(ones1[:], ones1_d[:])
        nc.sync.dma_start(ident[:], ident_d[:])
        nc.sync.dma_start(neglam[:], neglam_d[:])
        nc.sync.dma_start(tsc[:], tsc_d[:])
        nc.sync.dma_start(epsd[:], epsd_d[:])
        # big xs8 DMA last (split in two so row 0..67 lands first)
        xs8 = cst.tile([96, 2, H + 2, W], fp8, name="xs8", tag="xs8")
        xs_flat = xs8[:].rearrange("p a b c -> p (a b c)")
        nc.sync.dma_start(xs_flat[:, 0:XJ], xs_d[:, 0:XJ])
        nc.sync.dma_start(xs_flat[:, XJ:2 * XJ], xs_d[:, XJ:2 * XJ])

        dwv_res = [res.tile([96, N], bf16, name=f"dwv{i}", tag=f"dwv{i}")
                   for i in range(2)]

        smx = ctx.enter_context(tc.tile_pool(name="smx", bufs=1))

        # ================= PHASE A =================
        n_blk_total = H

        pa_stack = ExitStack()
        stg = pa_stack.enter_context(tc.tile_pool(name="stg", bufs=2))
        tro = pa_stack.enter_context(tc.tile_pool(name="tro", bufs=1))
        cvps = pa_stack.enter_context(
            tc.tile_pool(name="cvps", bufs=1, space="PSUM"))
        scps = pa_stack.enter_context(
            tc.tile_pool(name="scps", bufs=1, space="PSUM"))

        psc = scps.tile([96, 2, 96], f32, name="psc", tag="psc")
        blk_count = [0, 0]

        # ---- PE warm-up: dummy matmuls on the (small, early) weight
        # tiles while the 3.2MB xs8 DMA streams in; keeps HAM at 8/8.
        warm = cvps.tile([128, 4, 128], f32, name="cv0", tag="cv0")
        wflat = wt[0][:].rearrange("p a b c -> p (a b c)")
        wout = warm[0:96, :, :].rearrange("p a b -> p (a b)")
        for _ in range(72):
            nc.tensor.matmul(wout[:, 0:480], wt[0][:, 0, 0, 0:96],
                             wflat[:, 0:480], start=True, stop=True,
                             skip_group_check=True)

        def emit_scores(trts, nrows):
            for hf in range(2):
                for blk in range(nrows):
                    nc.tensor.matmul(
                        psc[:, hf, :],
                        trts[("q", hf)][:, blk, :],
                        trts[("k", hf)][:, blk, :],
                        start=(blk_count[hf] == 0),
                        stop=(blk_count[hf] == n_blk_total - 1),
                        skip_group_check=True)
                    blk_count[hf] += 1

        pending_scores = []
        ecnt = [0]

        for (r0, nw, cw0) in ROTS:
            stgs = {}
            for (p, hf) in (("q", 0), ("q", 1), ("k", 0), ("k", 1)):
                for ci, ncw in ((0, cw0), (1, nw - cw0)):
                    stgs[(p, hf, ci)] = stg.tile(
                        [96, 4 * ncw, 128], bf16,
                        name=f"s{p}{hf}{ci}", tag=f"s{p}{hf}{ci}")
            for g in range(5):
                Mg = GROUP_M[g]
                pst = [cvps.tile([128, 4, 128], f32, name=f"cv{w}",
                                 tag=f"cv{w}") for w in range(nw)]
                for ti, t in enumerate(TAP_ORDER):
                    oy, ox = OFFS[t]
                    (a0, a1), (b0, b1) = xr(ox)
                    for w in range(nw):
                        rw = r0 + 4 * w
                        nc.tensor.matmul(
                            pst[w][0:Mg, :, a0:a1],
                            wt[g][:, t, :, :],
                            xs8[:, :, 1 + rw + oy:5 + rw + oy, b0:b1],
                            start=(ti == 0), stop=(ti == 8),
                            perf_mode=DR,
                            skip_group_check=True)
                # evacuate, splitting psum partitions by segment.
                # HW rule: a partition access starting at base b may span at
                # most 128 (b=0), 64 (b=64), else 32 partitions - split
                # pieces to respect both src and dst bases.
                def _allowed(bp):
                    if bp == 0:
                        return 128
                    if bp % 64 == 0:
                        return 64
                    return 32

                for w in range(nw):
                    rloc = 4 * w          # rows within rotation
                    for (dst_key, plo, phi, olo) in GROUP_SEGS[g]:
                        (p, hf) = dst_key
                        cur = plo
                        while cur < phi:
                            od = olo + (cur - plo)
                            npart = min(phi - cur, _allowed(cur),
                                        _allowed(od))
                            src = pst[w][cur:cur + npart, :, :]
                            if p == "v":
                                seg = (r0 + rloc) * W
                                dst = dwv_res[hf][od:od + npart,
                                                  seg:seg + 512]\
                                    .rearrange("p (r x) -> p r x", x=128)
                            else:
                                ci = 0 if w < cw0 else 1
                                wloc = rloc - (0 if w < cw0 else 4 * cw0)
                                dst = stgs[(p, hf, ci)][od:od + npart,
                                                        wloc:wloc + 4, :]
                            if ecnt[0] % 2 == 0:
                                nc.scalar.copy(dst, src)
                            else:
                                nc.vector.tensor_copy(dst, src)
                            ecnt[0] += 1
                            cur += npart
            # DMA transposes for this rotation's q/k chunks
            for ci, ncw in ((0, cw0), (1, nw - cw0)):
                tr = {}
                for (p, hf) in (("q", 0), ("q", 1), ("k", 0), ("k", 1)):
                    tt = tro.tile([128, 4 * ncw, 96], bf16,
                                  name=f"t{p}{hf}{ci}", tag=f"t{p}{hf}{ci}")
                    nc.sync.dma_start_transpose(
                        tt[:], stgs[(p, hf, ci)][:].rearrange(
                            "p r x -> p (r x)"))
                    tr[(p, hf)] = tt
                pending_scores.append((tr, 4 * ncw))
            while len(pending_scores) > 2:
                emit_scores(*pending_scores.pop(0))
        while pending_scores:
            emit_scores(*pending_scores.pop(0))

        # ================= PHASE B: softmax + attn =================
        ex = []
        rr_ = []
        for hf in range(2):
            scl = smx.tile([96, 96], f32, name=f"scl{hf}", tag=f"scl{hf}")
            nc.vector.tensor_scalar(scl[:], psc[:, hf, :], tsc[:, hf:hf + 1],
                                    None, OP.mult)
            nm = smx.tile([96, 1], f32, name=f"nm{hf}", tag=f"nm{hf}")
            nc.vector.tensor_reduce(nm[:], scl[:], AX.X, OP.max, negate=True)
            e = smx.tile([96, 96], f32, name=f"e{hf}", tag=f"e{hf}")
            nc.scalar.activation(e[:], scl[:], AF.Exp, bias=nm[:, 0:1])
            sm = smx.tile([96, 1], f32, name=f"sm{hf}", tag=f"sm{hf}")
            nc.vector.tensor_reduce(sm[:], e[:], AX.X, OP.add)
            r = smx.tile([96, 1], f32, name=f"r{hf}", tag=f"r{hf}")
            nc.vector.reciprocal(r[:], sm[:])
            ex.append(e)
            rr_.append(r)
        pa_stack.close()

        atstack = ExitStack()
        atps = atstack.enter_context(
            tc.tile_pool(name="atps", bufs=1, space="PSUM"))
        r2n = smx.tile([96, 1], f32, name="r2n", tag="r2n")
        nc.vector.tensor_scalar(r2n[:], rr_[1][:], neglam[0:96, 0:1],
                                None, OP.mult)
        a1 = smx.tile([96, 96], f32, name="a1", tag="a1")
        nc.scalar.mul(a1[:], ex[0][:], rr_[0][:, 0:1])
        attn = smx.tile([96, 96], bf16, name="attn", tag="attn")
        nc.vector.scalar_tensor_tensor(attn[:], ex[1][:], r2n[:, 0:1],
                                       a1[:], OP.mult, OP.add)
        pt = atps.tile([96, 96], bf16, name="pt", tag="pt")
        nc.tensor.transpose(pt[:], attn[:], ident[:])
        attnT = smx.tile([96, 96], bf16, name="attnT", tag="attnT")
        nc.scalar.copy(attnT[:], pt[:])
        atstack.close()

        # ================= PHASE C (5-stage pipeline) =================
        with tc.tile_pool(name="yp", bufs=4) as yp, \
             tc.tile_pool(name="op_", bufs=2) as op_, \
             tc.tile_pool(name="yps", bufs=2, space="PSUM") as yps, \
             tc.tile_pool(name="sqps", bufs=1, space="PSUM") as sqps, \
             tc.tile_pool(name="rbps", bufs=1, space="PSUM") as rbps, \
             tc.tile_pool(name="ops", bufs=1, space="PSUM") as ops:
            py2 = {}
            ysb = {}
            yy2 = {}
            pss = {}
            rsb = {}
            prb = {}
            rbsb = {}
            ys8 = {}
            po2 = {}
            for it in range(NCH + 4):
                cA = it            # stage A chunk
                cB = it - 1
                cC = it - 2
                cD = it - 3
                cE = it - 4
                if cA < NCH:
                    seg = cA * CH
                    py2[cA] = yps.tile([96, 2, CH], f32, name="y2",
                                       tag="y2")
                    for hf in range(2):
                        nc.tensor.matmul(py2[cA][:, hf, :], attnT[:],
                                         dwv_res[hf][:, seg:seg + CH],
                                         start=True, stop=True,
                                         skip_group_check=True)
                    ysb[cA] = yp.tile([96, 2, CH], bf16, name="ysb",
                                      tag="ysb")
                    nc.scalar.copy(ysb[cA][:], py2[cA][:])
                if 0 <= cB < NCH:
                    yy2[cB] = yp.tile([96, 2, CH], bf16, name="yy2",
                                      tag="yy2")
                    nc.gpsimd.tensor_tensor(yy2[cB][:], ysb[cB][:],
                                            ysb[cB][:], OP.mult)
                if 0 <= cC < NCH:
                    pss[cC] = sqps.tile([1, CH], f32, name="ss", tag="ss")
                    nc.tensor.matmul(pss[cC][:], w96[:], yy2[cC][:, 0, :],
                                     start=True, stop=False,
                                     skip_group_check=True)
                    nc.tensor.matmul(pss[cC][:], w96[:], yy2[cC][:, 1, :],
                                     start=False, stop=True,
                                     skip_group_check=True)
                    rsb[cC] = op_.tile([1, CH], bf16, name="rs", tag="rs")
                    nc.scalar.activation(rsb[cC][:], pss[cC][:],
                                         AF.Abs_reciprocal_sqrt,
                                         bias=epsd[0:1, 0:1],
                                         scale=1.0 / 192.0)
                    del yy2[cC]
                if 0 <= cD < NCH:
                    prb[cD] = rbps.tile([96, CH], f32, name="rb", tag="rb")
                    nc.tensor.matmul(prb[cD][:], ones1[:], rsb[cD][:],
                                     start=True, stop=True,
                                     skip_group_check=True)
                    rbsb[cD] = op_.tile([96, CH], f32, name="rbs",
                                        tag="rbs")
                    nc.scalar.copy(rbsb[cD][:], prb[cD][:])
                    ys8[cD] = yp.tile([96, 2, CH], fp8, name="ys8",
                                      tag="ys8")
                    rb3 = rbsb[cD][:].rearrange("p (o n) -> p o n", o=1)\
                        .broadcast_to([96, 2, CH])
                    nc.vector.tensor_tensor(ys8[cD][:], ysb[cD][:], rb3,
                                            OP.mult)
                    del rsb[cD], prb[cD], ysb[cD]
                if 0 <= cE < NCH:
                    seg = cE * CH
                    po2[cE] = ops.tile([96, 2, CH], f32, name="po",
                                       tag="po")
                    for mt in range(2):
                        nc.tensor.matmul(po2[cE][:, mt, :], wo8[mt][:],
                                         ys8[cE][:], start=True, stop=True,
                                         perf_mode=DR,
                                         skip_group_check=True)
                    osb = op_.tile([96, 2, CH], f32, name="os", tag="os")
                    nc.vector.tensor_copy(osb[:], po2[cE][:])
                    for mt in range(2):
                        nc.sync.dma_start(
                            out_d[mt * 96:(mt + 1) * 96, seg:seg + CH],
                            osb[:, mt, :])
                    del ys8[cE], rbsb[cE], po2[cE]
                    py2.pop(cE, None)
    nc.compile()
    return nc


def _pow2_scale(maxabs, target=128.0):
    if maxabs <= 0:
        return 1.0
    return float(2.0 ** np.floor(np.log2(target / maxabs)))


def _prep_inputs(inputs):
    x = np.asarray(inputs["x"], np.float32)
    norm_w = np.asarray(inputs["norm_w"], np.float32)
    Wq = np.asarray(inputs["Wq"], np.float32)
    Wk = np.asarray(inputs["Wk"], np.float32)
    Wv = np.asarray(inputs["Wv"], np.float32)
    Dq = np.asarray(inputs["Dq"], np.float32)
    Dk = np.asarray(inputs["Dk"], np.float32)
    Dv = np.asarray(inputs["Dv"], np.float32)
    t1 = np.asarray(inputs["t1"], np.float32)
    t2 = np.asarray(inputs["t2"], np.float32)
    hn_w = np.asarray(inputs["hn_w"], np.float32)
    Wo = np.asarray(inputs["Wo"], np.float32)
    lam = float(np.exp(np.sum(inputs["lq1"] * inputs["lk1"],
                              dtype=np.float64))
                - np.exp(np.sum(inputs["lq2"] * inputs["lk2"],
                                dtype=np.float64))
                + LAM_INIT)

    var = x.var(axis=1)
    s = 1.0 / np.sqrt(var + 1e-5)
    xs = (x * s[:, None, :, :]).reshape(B, C, H, W)

    Wf = {"q": Wq * norm_w[None, :], "k": Wk * norm_w[None, :],
          "v": Wv * norm_w[None, :]}
    Dd = {"q": Dq, "k": Dk, "v": Dv}

    in_maps = []
    so_list = []
    for core in range(8):
        b, h = core // 2, core % 2
        sl = slice(h * 192, (h + 1) * 192)
        m = {}
        xpad = np.zeros((96, 2, H + 2, W), np.float32)
        xc = xs[b]
        xpad[:, 0, 1:H + 1, :] = xc[0:96]
        xpad[:, 1, 1:H + 1, :] = xc[96:192]
        m["xs8"] = np.clip(xpad, -224, 224).astype(FP8).reshape(96, -1)

        K3s = {}
        scales = {}
        for nm in ("q", "k", "v"):
            Wh = Wf[nm][sl]
            dh = Dd[nm][sl, 0].reshape(192, 9)
            K3 = Wh[:, :, None] * dh[:, None, :]   # [192 o, 192 cg, 9 t]
            sp = _pow2_scale(np.abs(K3).max())
            scales[nm] = sp
            K3s[nm] = np.clip(K3 * sp, -224, 224)

        # M-group packed conv weights
        tnames = ["q", "q", "k", "k", "v", "v"]
        for g in range(5):
            Mg = GROUP_M[g]
            rows = []
            for mm_ in range(Mg):
                u = 128 * g + mm_
                tname = tnames[u // 96]
                rows.append(K3s[tname][(u % 192)])
            blk = np.stack(rows)                  # [Mg, 192 cg, 9 t]
            w4 = blk.reshape(Mg, 2, 96, 9)        # [m, j, c, t]
            m[f"wg{g}"] = np.ascontiguousarray(
                w4.transpose(2, 3, 1, 0)          # [c, t, j, m]
            ).astype(FP8).reshape(96, -1)

        th = np.array([t1[h, 0, 0], t2[h, 0, 0]], np.float32)
        m["tsc"] = np.broadcast_to(
            (th / (scales["q"] * scales["k"]))[None, :], (96, 2)
        ).astype(np.float32).copy()

        # out-projection (fp8 DR, r folded into ys8 = 16*yhat)
        Wo_hf = Wo[:, sl] * (hn_w[h] * (1.0 - LAM_INIT))[None, :]
        lhsT = Wo_hf.T.astype(np.float32)         # [192 y-ch, 192 out]
        s_o = _pow2_scale(np.abs(lhsT).max() / 16.0)
        lw = np.clip(lhsT * (s_o / 16.0), -448, 448)
        lw = lw.reshape(2, 96, 2, 96)             # [j, c, mt, o]
        for mt in range(2):
            m[f"wo8_{mt}"] = np.ascontiguousarray(
                lw[:, :, mt, :].transpose(1, 0, 2)
            ).astype(FP8).reshape(96, -1)
        so_list.append(s_o)

        sv = scales["v"]
        m["epsd"] = np.full((1, 1), 1e-6 * sv * sv / 256.0, np.float32)
        m["w96"] = np.full((96, 1), 1.0 / 256.0, BF16)
        m["ones1"] = np.ones((1, 96), BF16)
        m["ident"] = np.eye(96, dtype=BF16)
        m["neglam"] = np.full((128, 1), -lam, np.float32)
        in_maps.append(m)
    return in_maps, so_list


def kernel(**inputs):
    from concourse import bass_utils

    if "nc" not in _CACHED:
        _CACHED["nc"] = _build_program()
    nc = _CACHED["nc"]

    in_maps, so_list = _prep_inputs(inputs)
    results = bass_utils.run_bass_kernel_spmd(
        nc, in_maps, core_ids=list(range(8))).results

    x = np.asarray(inputs["x"], np.float32)
    out = np.empty((B, C, N), np.float32)
    for b in range(B):
        out[b] = (results[2 * b]["out"] / so_list[2 * b]
                  + results[2 * b + 1]["out"] / so_list[2 * b + 1])
    out = out.reshape(B, C, H, W) + x
    return out.astype(np.float32)


# revision 15
# speedup vs baseline: 1.5578x; 1.1727x over previous
"""Trainium2 Bass kernel for nn_DTAM (differential transposed-attention module).

Sharding: 8 cores = batch(4) x head(2). Each core computes its (b, h) shard
end-to-end; host does LayerNorm scale precompute, weight folding, and the
final partial-sum + residual merge (including per-core fp8 descale).

v4 design (vs 571us v2 baseline):
- Dense fused 3x3 conv (fp8 DoubleRow) with M=128 OUTPUT PACKING: the 576
  output channels (q/k/v x 2 halves x 96) stream as 5 M-groups
  (4x128 + 1x64) instead of 6x96. The moving-operand port (2B/part/cyc)
  is the hard wall, so fewer streams = directly less PE time.
  Evacuation uses partition-shifted engine copies (PSUM part p -> SBUF
  part q, p != q), verified on HW.
- 4-row conv windows: each matmul streams 512 px (1016-elem moving AP),
  7 PSUM banks rotate per 28 rows; the 8th bank holds the persistent
  score accumulator psc (one accumulation chain per hf over all 128
  row-blocks).
- PE warm-up dummy matmuls run while the 3.2MB xs8 DMA lands, keeping the
  HAM clock-gate at full rate for the first conv rotation.
- Phase C is software-pipelined 5 stages deep across 512-px chunks, so
  the in-chunk PE->ACT->PE->...->DVE dependency chain never stalls PE:
    A: y matmuls (PE), ysb=cast (ACT)
    B: yy=ysb^2 (GPSIMD)
    C: stats matmul (PE), rsqrt (ACT)
    D: broadcast matmul (PE), rbsb copy (ACT), ys8=ysb*r fp8 (DVE)
    E: out-proj fp8-DoubleRow (PE), osb cast (DVE), DMA out
  RMS-norm r is folded into the fp8 proj input (|16*yhat| <= 222 < 448,
  mathematically bounded, no clipping); all scales are pow2-exact, and
  the final pow2 descale happens on host during the partial-sum merge.
"""

import numpy as np
import ml_dtypes
from contextlib import ExitStack

BF16 = ml_dtypes.bfloat16
FP8 = ml_dtypes.float8_e4m3fn

# ---- problem constants (hardcoded per contest rules) ----
B, C, H, W = 4, 192, 128, 128
HEADS = 2
N = H * W                 # 16384
HC = 96
LAM_INIT = 0.8
CH = 512                  # phase C chunk (px)
NCH = N // CH             # 32 chunks

# conv rotations: (start_row, n_windows, windows_in_first_transpose_chunk)
ROTS = [(0, 7, 4), (28, 7, 4), (56, 7, 4), (84, 7, 4), (112, 4, 2)]

# M-group packing of the 576 output channels, order q0,q1,k0,k1,v0,v1.
# Group g covers global channels [128g, 128g+Mg); segment list maps psum
# partition ranges to (tensor, hf, dest channel offset).
GROUP_M = [128, 128, 128, 128, 64]
GROUP_SEGS = [
    [(("q", 0), 0, 96, 0), (("q", 1), 96, 128, 0)],
    [(("q", 1), 0, 64, 32), (("k", 0), 64, 128, 0)],
    [(("k", 0), 0, 32, 64), (("k", 1), 32, 128, 0)],
    [(("v", 0), 0, 96, 0), (("v", 1), 96, 128, 0)],
    [(("v", 1), 0, 64, 32)],
]

_CACHED = {}


def _build_program():
    import concourse.bass as bass
    import concourse.bacc as bacc
    import concourse.tile as tile
    from concourse import mybir

    f32 = mybir.dt.float32
    bf16 = mybir.dt.bfloat16
    fp8 = mybir.dt.float8e4
    AF = mybir.ActivationFunctionType
    OP = mybir.AluOpType
    AX = mybir.AxisListType
    DR = mybir.MatmulPerfMode.DoubleRow

    nc = bacc.Bacc("TRN2", target_bir_lowering=False, debug=False,
                   num_devices=8)

    # ---- DRAM I/O ----
    XJ = (H + 2) * W
    xs_d = nc.dram_tensor("xs8", [96, 2 * XJ], fp8, kind="ExternalInput")
    wg_d = [nc.dram_tensor(f"wg{g}", [96, 9 * 2 * GROUP_M[g]], fp8,
                           kind="ExternalInput") for g in range(5)]
    wo8_d = [nc.dram_tensor(f"wo8_{mt}", [96, 2 * 96], fp8,
                            kind="ExternalInput") for mt in range(2)]
    w96_d = nc.dram_tensor("w96", [96, 1], bf16, kind="ExternalInput")
    ones1_d = nc.dram_tensor("ones1", [1, 96], bf16, kind="ExternalInput")
    ident_d = nc.dram_tensor("ident", [96, 96], bf16, kind="ExternalInput")
    neglam_d = nc.dram_tensor("neglam", [128, 1], f32, kind="ExternalInput")
    tsc_d = nc.dram_tensor("tsc", [96, 2], f32, kind="ExternalInput")
    epsd_d = nc.dram_tensor("epsd", [1, 1], f32, kind="ExternalInput")
    out_d = nc.dram_tensor("out", [192, N], f32, kind="ExternalOutput")

    OFFS = [(t // 3 - 1, t % 3 - 1) for t in range(9)]
    TAP_ORDER = [4] + [t for t in range(9) if t != 4]

    def xr(ox):
        if ox == -1:
            return (1, 128), (0, 127)
        if ox == 1:
            return (0, 127), (1, 128)
        return (0, 128), (0, 128)

    with tile.TileContext(nc) as tc, ExitStack() as ctx:
        cst = ctx.enter_context(tc.tile_pool(name="cst", bufs=1))
        res = ctx.enter_context(tc.tile_pool(name="res", bufs=1))

        # ---- constants: small DMAs first so warm-up can start early ----
        wt = []
        for g in range(5):
            t = cst.tile([96, 9, 2, GROUP_M[g]], fp8, name=f"wg{g}",
                         tag=f"wg{g}")
            nc.sync.dma_start(t[:].rearrange("p a b c -> p (a b c)"),
                              wg_d[g][:])
            wt.append(t)
        wo8 = []
        for mt in range(2):
            t = cst.tile([96, 2, 96], fp8, name=f"wo8{mt}", tag=f"wo8{mt}")
            nc.sync.dma_start(t[:].rearrange("p a b -> p (a b)"),
                              wo8_d[mt][:])
            wo8.append(t)
        w96 = cst.tile([96, 1], bf16, name="w96", tag="w96")
        ones1 = cst.tile([1, 96], bf16, name="o1", tag="o1")
        ident = cst.tile([96, 96], bf16, name="id", tag="id")
        neglam = cst.tile([128, 1], f32, name="nl", tag="nl")
        tsc = cst.tile([96, 2], f32, name="tsc", tag="tsc")
        epsd = cst.tile([1, 1], f32, name="epsd", tag="epsd")
        nc.sync.dma_start(w96[:], w96_d[:])
        nc.sync.dma_start(ones1[:], ones1_d[:])
        nc.sync.dma_start(ident[:], ident_d[:])
        nc.sync.dma_start(neglam[:], neglam_d[:])
        nc.sync.dma_start(tsc[:], tsc_d[:])
        nc.sync.dma_start(epsd[:], epsd_d[:])
        # big xs8 DMA last (split in two so row 0..67 lands first)
        xs8 = cst.tile([96, 2, H + 2, W], fp8, name="xs8", tag="xs8")
        xs_flat = xs8[:].rearrange("p a b c -> p (a b c)")
        nc.sync.dma_start(xs_flat[:, 0:XJ], xs_d[:, 0:XJ])
        nc.sync.dma_start(xs_flat[:, XJ:2 * XJ], xs_d[:, XJ:2 * XJ])

        dwv_res = [res.tile([96, N], bf16, name=f"dwv{i}", tag=f"dwv{i}")
                   for i in range(2)]

        smx = ctx.enter_context(tc.tile_pool(name="smx", bufs=1))

        # ================= PHASE A =================
        n_blk_total = H

        pa_stack = ExitStack()
        stg = pa_stack.enter_context(tc.tile_pool(name="stg", bufs=2))
        tro = pa_stack.enter_context(tc.tile_pool(name="tro", bufs=1))
        cvps = pa_stack.enter_context(
            tc.tile_pool(name="cvps", bufs=1, space="PSUM"))
        scps = pa_stack.enter_context(
            tc.tile_pool(name="scps", bufs=1, space="PSUM"))

        psc = scps.tile([96, 2, 96], f32, name="psc", tag="psc")
        blk_count = [0, 0]

        # ---- PE warm-up: dummy matmuls on the (small, early) weight
        # tiles while the 3.2MB xs8 DMA streams in; keeps HAM at 8/8.
        warm = cvps.tile([128, 4, 128], f32, name="cv0", tag="cv0")
        wflat = wt[0][:].rearrange("p a b c -> p (a b c)")
        wout = warm[0:96, :, :].rearrange("p a b -> p (a b)")
        for _ in range(72):
            nc.tensor.matmul(wout[:, 0:480], wt[0][:, 0, 0, 0:96],
                             wflat[:, 0:480], start=True, stop=True,
                             skip_group_check=True)

        def emit_scores(trts, nrows):
            for hf in range(2):
                for blk in range(nrows):
                    nc.tensor.matmul(
                        psc[:, hf, :],
                        trts[("q", hf)][:, blk, :],
                        trts[("k", hf)][:, blk, :],
                        start=(blk_count[hf] == 0),
                        stop=(blk_count[hf] == n_blk_total - 1),
                        skip_group_check=True)
                    blk_count[hf] += 1

        pending_scores = []
        ecnt = [0]

        for (r0, nw, cw0) in ROTS:
            stgs = {}
            for (p, hf) in (("q", 0), ("q", 1), ("k", 0), ("k", 1)):
                for ci, ncw in ((0, cw0), (1, nw - cw0)):
                    stgs[(p, hf, ci)] = stg.tile(
                        [96, 4 * ncw, 128], bf16,
                        name=f"s{p}{hf}{ci}", tag=f"s{p}{hf}{ci}")
            for g in range(5):
                Mg = GROUP_M[g]
                pst = [cvps.tile([128, 4, 128], f32, name=f"cv{w}",
                                 tag=f"cv{w}") for w in range(nw)]
                for ti, t in enumerate(TAP_ORDER):
                    oy, ox = OFFS[t]
                    (a0, a1), (b0, b1) = xr(ox)
                    for w in range(nw):
                        rw = r0 + 4 * w
                        nc.tensor.matmul(
                            pst[w][0:Mg, :, a0:a1],
                            wt[g][:, t, :, :],
                            xs8[:, :, 1 + rw + oy:5 + rw + oy, b0:b1],
                            start=(ti == 0), stop=(ti == 8),
                            perf_mode=DR,
                            skip_group_check=True)
                # evacuate, splitting psum partitions by segment.
                # HW rule: a partition access starting at base b may span at
                # most 128 (b=0), 64 (b=64), else 32 partitions - split
                # pieces to respect both src and dst bases.
                def _allowed(bp):
                    if bp == 0:
                        return 128
                    if bp % 64 == 0:
                        return 64
                    return 32

                for w in range(nw):
                    rloc = 4 * w          # rows within rotation
                    for (dst_key, plo, phi, olo) in GROUP_SEGS[g]:
                        (p, hf) = dst_key
                        cur = plo
                        while cur < phi:
                            od = olo + (cur - plo)
                            npart = min(phi - cur, _allowed(cur),
                                        _allowed(od))
                            src = pst[w][cur:cur + npart, :, :]
                            if p == "v":
                                seg = (r0 + rloc) * W
                                dst = dwv_res[hf][od:od + npart,
                                                  seg:seg + 512]\
                                    .rearrange("p (r x) -> p r x", x=128)
                            else:
                                ci = 0 if w < cw0 else 1
                                wloc = rloc - (0 if w < cw0 else 4 * cw0)
                                dst = stgs[(p, hf, ci)][od:od + npart,
                                                        wloc:wloc + 4, :]
                            if ecnt[0] % 2 == 0:
                                nc.scalar.copy(dst, src)
                            else:
                                nc.vector.tensor_copy(dst, src)
                            ecnt[0] += 1
                            cur += npart
            # DMA transposes for this rotation's q/k chunks
            for ci, ncw in ((0, cw0), (1, nw - cw0)):
                tr = {}
                for (p, hf) in (("q", 0), ("q", 1), ("k", 0), ("k", 1)):
                    tt = tro.tile([128, 4 * ncw, 96], bf16,
                                  name=f"t{p}{hf}{ci}", tag=f"t{p}{hf}{ci}")
                    nc.sync.dma_start_transpose(
                        tt[:], stgs[(p, hf, ci)][:].rearrange(
                            "p r x -> p (r x)"))
                    tr[(p, hf)] = tt
                pending_scores.append((tr, 4 * ncw))
            while len(pending_scores) > 2:
                emit_scores(*pending_scores.pop(0))
        while pending_scores:
            emit_scores(*pending_scores.pop(0))

        # ================= PHASE B: softmax + attn =================
        ex = []
        rr_ = []
        for hf in range(2):
            scl = smx.tile([96, 96], f32, name=f"scl{hf}", tag=f"scl{hf}")
            nc.vector.tensor_scalar(scl[:], psc[:, hf, :], tsc[:, hf:hf + 1],
                                    None, OP.mult)
            nm = smx.tile([96, 1], f32, name=f"nm{hf}", tag=f"nm{hf}")
            nc.vector.tensor_reduce(nm[:], scl[:], AX.X, OP.max, negate=True)
            e = smx.tile([96, 96], f32, name=f"e{hf}", tag=f"e{hf}")
            nc.scalar.activation(e[:], scl[:], AF.Exp, bias=nm[:, 0:1])
            sm = smx.tile([96, 1], f32, name=f"sm{hf}", tag=f"sm{hf}")
            nc.vector.tensor_reduce(sm[:], e[:], AX.X, OP.add)
            r = smx.tile([96, 1], f32, name=f"r{hf}", tag=f"r{hf}")
            nc.vector.reciprocal(r[:], sm[:])
            ex.append(e)
            rr_.append(r)
        pa_stack.close()

        atstack = ExitStack()
        atps = atstack.enter_context(
            tc.tile_pool(name="atps", bufs=1, space="PSUM"))
        r2n = smx.tile([96, 1], f32, name="r2n", tag="r2n")
        nc.vector.tensor_scalar(r2n[:], rr_[1][:], neglam[0:96, 0:1],
                                None, OP.mult)
        a1 = smx.tile([96, 96], f32, name="a1", tag="a1")
        nc.scalar.mul(a1[:], ex[0][:], rr_[0][:, 0:1])
        attn = smx.tile([96, 96], bf16, name="attn", tag="attn")
        nc.vector.scalar_tensor_tensor(attn[:], ex[1][:], r2n[:, 0:1],
                                       a1[:], OP.mult, OP.add)
        pt = atps.tile([96, 96], bf16, name="pt", tag="pt")
        nc.tensor.transpose(pt[:], attn[:], ident[:])
        attnT = smx.tile([96, 96], bf16, name="attnT", tag="attnT")
        nc.scalar.copy(attnT[:], pt[:])
        atstack.close()

        # ================= PHASE C (5-stage pipeline) =================
        with tc.tile_pool(name="yp", bufs=4) as yp, \
             tc.tile_pool(name="op_", bufs=2) as op_, \
             tc.tile_pool(name="yps", bufs=2, space="PSUM") as yps, \
             tc.tile_pool(name="sqps", bufs=1, space="PSUM") as sqps, \
             tc.tile_pool(name="rbps", bufs=1, space="PSUM") as rbps, \
             tc.tile_pool(name="ops", bufs=1, space="PSUM") as ops:
            py2 = {}
            ysb = {}
            yy2 = {}
            pss = {}
            rsb = {}
            prb = {}
            rbsb = {}
            ys8 = {}
            po2 = {}
            for it in range(NCH + 4):
                cA = it            # stage A chunk
                cB = it - 1
                cC = it - 2
                cD = it - 3
                cE = it - 4
                if cA < NCH:
                    seg = cA * CH
                    py2[cA] = yps.tile([96, 2, CH], f32, name="y2",
                                       tag="y2")
                    for hf in range(2):
                        nc.tensor.matmul(py2[cA][:, hf, :], attnT[:],
                                         dwv_res[hf][:, seg:seg + CH],
                                         start=True, stop=True,
                                         skip_group_check=True)
                    ysb[cA] = yp.tile([96, 2, CH], bf16, name="ysb",
                                      tag="ysb")
                    nc.scalar.copy(ysb[cA][:], py2[cA][:])
                if 0 <= cB < NCH:
                    yy2[cB] = yp.tile([96, 2, CH], bf16, name="yy2",
                                      tag="yy2")
                    nc.vector.tensor_tensor(yy2[cB][:], ysb[cB][:],
                                            ysb[cB][:], OP.mult)
                if 0 <= cC < NCH:
                    pss[cC] = sqps.tile([1, CH], f32, name="ss", tag="ss")
                    nc.tensor.matmul(pss[cC][:], w96[:], yy2[cC][:, 0, :],
                                     start=True, stop=False,
                                     skip_group_check=True)
                    nc.tensor.matmul(pss[cC][:], w96[:], yy2[cC][:, 1, :],
                                     start=False, stop=True,
                                     skip_group_check=True)
                    rsb[cC] = op_.tile([1, CH], bf16, name="rs", tag="rs")
                    nc.scalar.activation(rsb[cC][:], pss[cC][:],
                                         AF.Abs_reciprocal_sqrt,
                                         bias=epsd[0:1, 0:1],
                                         scale=1.0 / 192.0)
                    del yy2[cC]
                if 0 <= cD < NCH:
                    prb[cD] = rbps.tile([96, CH], f32, name="rb", tag="rb")
                    nc.tensor.matmul(prb[cD][:], ones1[:], rsb[cD][:],
                                     start=True, stop=True,
                                     skip_group_check=True)
                    rbsb[cD] = op_.tile([96, CH], bf16, name="rbs",
                                        tag="rbs")
                    nc.vector.tensor_copy(rbsb[cD][:], prb[cD][:])
                    ys8[cD] = yp.tile([96, 2, CH], fp8, name="ys8",
                                      tag="ys8")
                    rb3 = rbsb[cD][:].rearrange("p (o n) -> p o n", o=1)\
                        .broadcast_to([96, 2, CH])
                    nc.vector.tensor_tensor(ys8[cD][:], ysb[cD][:], rb3,
                                            OP.mult)
                    del rsb[cD], prb[cD], ysb[cD]
                if 0 <= cE < NCH:
                    seg = cE * CH
                    po2[cE] = ops.tile([96, 2, CH], f32, name="po",
                                       tag="po")
                    for mt in range(2):
                        nc.tensor.matmul(po2[cE][:, mt, :], wo8[mt][:],
                                         ys8[cE][:], start=True, stop=True,
                                         perf_mode=DR,
                                         skip_group_check=True)
                    osb = op_.tile([96, 2, CH], f32, name="os", tag="os")
                    nc.scalar.copy(osb[:, 0, :], po2[cE][:, 0, :])
                    nc.vector.tensor_copy(osb[:, 1, :], po2[cE][:, 1, :])
                    for mt in range(2):
                        nc.sync.dma_start(
                            out_d[mt * 96:(mt + 1) * 96, seg:seg + CH],
                            osb[:, mt, :])
                    del ys8[cE], rbsb[cE], po2[cE]
                    py2.pop(cE, None)
    nc.compile()
    return nc


def _pow2_scale(maxabs, target=128.0):
    if maxabs <= 0:
        return 1.0
    return float(2.0 ** np.floor(np.log2(target / maxabs)))


def _prep_inputs(inputs):
    x = np.asarray(inputs["x"], np.float32)
    norm_w = np.asarray(inputs["norm_w"], np.float32)
    Wq = np.asarray(inputs["Wq"], np.float32)
    Wk = np.asarray(inputs["Wk"], np.float32)
    Wv = np.asarray(inputs["Wv"], np.float32)
    Dq = np.asarray(inputs["Dq"], np.float32)
    Dk = np.asarray(inputs["Dk"], np.float32)
    Dv = np.asarray(inputs["Dv"], np.float32)
    t1 = np.asarray(inputs["t1"], np.float32)
    t2 = np.asarray(inputs["t2"], np.float32)
    hn_w = np.asarray(inputs["hn_w"], np.float32)
    Wo = np.asarray(inputs["Wo"], np.float32)
    lam = float(np.exp(np.sum(inputs["lq1"] * inputs["lk1"],
                              dtype=np.float64))
                - np.exp(np.sum(inputs["lq2"] * inputs["lk2"],
                                dtype=np.float64))
                + LAM_INIT)

    var = x.var(axis=1)
    s = 1.0 / np.sqrt(var + 1e-5)
    xs = (x * s[:, None, :, :]).reshape(B, C, H, W)

    Wf = {"q": Wq * norm_w[None, :], "k": Wk * norm_w[None, :],
          "v": Wv * norm_w[None, :]}
    Dd = {"q": Dq, "k": Dk, "v": Dv}

    in_maps = []
    so_list = []
    for core in range(8):
        b, h = core // 2, core % 2
        sl = slice(h * 192, (h + 1) * 192)
        m = {}
        xpad = np.zeros((96, 2, H + 2, W), np.float32)
        xc = xs[b]
        xpad[:, 0, 1:H + 1, :] = xc[0:96]
        xpad[:, 1, 1:H + 1, :] = xc[96:192]
        m["xs8"] = np.clip(xpad, -224, 224).astype(FP8).reshape(96, -1)

        K3s = {}
        scales = {}
        for nm in ("q", "k", "v"):
            Wh = Wf[nm][sl]
            dh = Dd[nm][sl, 0].reshape(192, 9)
            K3 = Wh[:, :, None] * dh[:, None, :]   # [192 o, 192 cg, 9 t]
            sp = _pow2_scale(np.abs(K3).max())
            scales[nm] = sp
            K3s[nm] = np.clip(K3 * sp, -224, 224)

        # M-group packed conv weights
        tnames = ["q", "q", "k", "k", "v", "v"]
        for g in range(5):
            Mg = GROUP_M[g]
            rows = []
            for mm_ in range(Mg):
                u = 128 * g + mm_
                tname = tnames[u // 96]
                rows.append(K3s[tname][(u % 192)])
            blk = np.stack(rows)                  # [Mg, 192 cg, 9 t]
            w4 = blk.reshape(Mg, 2, 96, 9)        # [m, j, c, t]
            m[f"wg{g}"] = np.ascontiguousarray(
                w4.transpose(2, 3, 1, 0)          # [c, t, j, m]
            ).astype(FP8).reshape(96, -1)

        th = np.array([t1[h, 0, 0], t2[h, 0, 0]], np.float32)
        m["tsc"] = np.broadcast_to(
            (th / (scales["q"] * scales["k"]))[None, :], (96, 2)
        ).astype(np.float32).copy()

        # out-projection (fp8 DR, r folded into ys8 = 16*yhat)
        Wo_hf = Wo[:, sl] * (hn_w[h] * (1.0 - LAM_INIT))[None, :]
        lhsT = Wo_hf.T.astype(np.float32)         # [192 y-ch, 192 out]
        s_o = _pow2_scale(np.abs(lhsT).max() / 16.0)
        lw = np.clip(lhsT * (s_o / 16.0), -448, 448)
        lw = lw.reshape(2, 96, 2, 96)             # [j, c, mt, o]
        for mt in range(2):
            m[f"wo8_{mt}"] = np.ascontiguousarray(
                lw[:, :, mt, :].transpose(1, 0, 2)
            ).astype(FP8).reshape(96, -1)
        so_list.append(s_o)

        sv = scales["v"]
        m["epsd"] = np.full((1, 1), 1e-6 * sv * sv / 256.0, np.float32)
        m["w96"] = np.full((96, 1), 1.0 / 256.0, BF16)
        m["ones1"] = np.ones((1, 96), BF16)
        m["ident"] = np.eye(96, dtype=BF16)
        m["neglam"] = np.full((128, 1), -lam, np.float32)
        in_maps.append(m)
    return in_maps, so_list


def kernel(**inputs):
    from concourse import bass_utils

    if "nc" not in _CACHED:
        _CACHED["nc"] = _build_program()
    nc = _CACHED["nc"]

    in_maps, so_list = _prep_inputs(inputs)
    results = bass_utils.run_bass_kernel_spmd(
        nc, in_maps, core_ids=list(range(8))).results

    x = np.asarray(inputs["x"], np.float32)
    out = np.empty((B, C, N), np.float32)
    for b in range(B):
        out[b] = (results[2 * b]["out"] / so_list[2 * b]
                  + results[2 * b + 1]["out"] / so_list[2 * b + 1])
    out = out.reshape(B, C, H, W) + x
    return out.astype(np.float32)


# revision 25
# speedup vs baseline: 1.5893x; 1.0202x over previous
"""Trainium2 Bass kernel for nn_DTAM (differential transposed-attention module).

Sharding: 8 cores = batch(4) x head(2). Each core computes its (b, h) shard
end-to-end; host does LayerNorm scale precompute, weight folding, and the
final partial-sum + residual merge (including per-core fp8 descale).

v4 design (vs 571us v2 baseline):
- Dense fused 3x3 conv (fp8 DoubleRow) with M=128 OUTPUT PACKING: the 576
  output channels (q/k/v x 2 halves x 96) stream as 5 M-groups
  (4x128 + 1x64) instead of 6x96. The moving-operand port (2B/part/cyc)
  is the hard wall, so fewer streams = directly less PE time.
  Evacuation uses partition-shifted engine copies (PSUM part p -> SBUF
  part q, p != q), verified on HW.
- 4-row conv windows: each matmul streams 512 px (1016-elem moving AP),
  7 PSUM banks rotate per 28 rows; the 8th bank holds the persistent
  score accumulator psc (one accumulation chain per hf over all 128
  row-blocks).
- PE warm-up dummy matmuls run while the 3.2MB xs8 DMA lands, keeping the
  HAM clock-gate at full rate for the first conv rotation.
- Phase C is software-pipelined 5 stages deep across 512-px chunks, so
  the in-chunk PE->ACT->PE->...->DVE dependency chain never stalls PE:
    A: y matmuls (PE), ysb=cast (ACT)
    B: yy=ysb^2 (GPSIMD)
    C: stats matmul (PE), rsqrt (ACT)
    D: broadcast matmul (PE), rbsb copy (ACT), ys8=ysb*r fp8 (DVE)
    E: out-proj fp8-DoubleRow (PE), osb cast (DVE), DMA out
  RMS-norm r is folded into the fp8 proj input (|16*yhat| <= 222 < 448,
  mathematically bounded, no clipping); all scales are pow2-exact, and
  the final pow2 descale happens on host during the partial-sum merge.
"""

import numpy as np
import ml_dtypes
from contextlib import ExitStack

BF16 = ml_dtypes.bfloat16
FP8 = ml_dtypes.float8_e4m3fn

# ---- problem constants (hardcoded per contest rules) ----
B, C, H, W = 4, 192, 128, 128
HEADS = 2
N = H * W                 # 16384
HC = 96
LAM_INIT = 0.8
CH = 512                  # phase C chunk (px)
NCH = N // CH             # 32 chunks

# conv rotations: (start_row, n_windows, windows_in_first_transpose_chunk)
ROTS = [(0, 7, 4), (28, 7, 4), (56, 7, 4), (84, 7, 4), (112, 4, 2)]

# M-group packing of the 576 output channels, order q0,q1,k0,k1,v0,v1.
# Group g covers global channels [128g, 128g+Mg); segment list maps psum
# partition ranges to (tensor, hf, dest channel offset).
GROUP_M = [128, 128, 128, 128, 64]
GROUP_SEGS = [
    [(("q", 0), 0, 96, 0), (("q", 1), 96, 128, 0)],
    [(("q", 1), 0, 64, 32), (("k", 0), 64, 128, 0)],
    [(("k", 0), 0, 32, 64), (("k", 1), 32, 128, 0)],
    [(("v", 0), 0, 96, 0), (("v", 1), 96, 128, 0)],
    [(("v", 1), 0, 64, 32)],
]

_CACHED = {}


def _build_program():
    import concourse.bass as bass
    import concourse.bacc as bacc
    import concourse.tile as tile
    from concourse import mybir

    f32 = mybir.dt.float32
    bf16 = mybir.dt.bfloat16
    fp8 = mybir.dt.float8e4
    AF = mybir.ActivationFunctionType
    OP = mybir.AluOpType
    AX = mybir.AxisListType
    DR = mybir.MatmulPerfMode.DoubleRow

    nc = bacc.Bacc("TRN2", target_bir_lowering=False, debug=False,
                   num_devices=8)

    # ---- DRAM I/O ----
    XJ = (H + 2) * W
    xs_d = nc.dram_tensor("xs8", [96, 2 * XJ], fp8, kind="ExternalInput")
    wg_d = [nc.dram_tensor(f"wg{g}", [96, 9 * 2 * GROUP_M[g]], fp8,
                           kind="ExternalInput") for g in range(5)]
    wo8_d = [nc.dram_tensor(f"wo8_{mt}", [96, 2 * 96], fp8,
                            kind="ExternalInput") for mt in range(2)]
    w96_d = nc.dram_tensor("w96", [96, 96], bf16, kind="ExternalInput")
    ident_d = nc.dram_tensor("ident", [96, 96], bf16, kind="ExternalInput")
    neglam_d = nc.dram_tensor("neglam", [128, 1], f32, kind="ExternalInput")
    tsc_d = nc.dram_tensor("tsc", [96, 2], f32, kind="ExternalInput")
    epsd_d = nc.dram_tensor("epsd", [96, 1], f32, kind="ExternalInput")
    out_d = nc.dram_tensor("out", [192, N], f32, kind="ExternalOutput")

    OFFS = [(t // 3 - 1, t % 3 - 1) for t in range(9)]
    TAP_ORDER = [4] + [t for t in range(9) if t != 4]

    def xr(ox):
        if ox == -1:
            return (1, 128), (0, 127)
        if ox == 1:
            return (0, 127), (1, 128)
        return (0, 128), (0, 128)

    with tile.TileContext(nc) as tc, ExitStack() as ctx:
        cst = ctx.enter_context(tc.tile_pool(name="cst", bufs=1))
        res = ctx.enter_context(tc.tile_pool(name="res", bufs=1))

        # ---- constants: small DMAs first so warm-up can start early ----
        wt = []
        for g in range(5):
            t = cst.tile([96, 9, 2, GROUP_M[g]], fp8, name=f"wg{g}",
                         tag=f"wg{g}")
            nc.sync.dma_start(t[:].rearrange("p a b c -> p (a b c)"),
                              wg_d[g][:])
            wt.append(t)
        wo8 = []
        for mt in range(2):
            t = cst.tile([96, 2, 96], fp8, name=f"wo8{mt}", tag=f"wo8{mt}")
            nc.sync.dma_start(t[:].rearrange("p a b -> p (a b)"),
                              wo8_d[mt][:])
            wo8.append(t)
        w96 = cst.tile([96, 96], bf16, name="w96", tag="w96")
        ident = cst.tile([96, 96], bf16, name="id", tag="id")
        neglam = cst.tile([128, 1], f32, name="nl", tag="nl")
        tsc = cst.tile([96, 2], f32, name="tsc", tag="tsc")
        epsd = cst.tile([96, 1], f32, name="epsd", tag="epsd")
        nc.sync.dma_start(w96[:], w96_d[:])
        nc.sync.dma_start(ident[:], ident_d[:])
        nc.sync.dma_start(neglam[:], neglam_d[:])
        nc.sync.dma_start(tsc[:], tsc_d[:])
        nc.sync.dma_start(epsd[:], epsd_d[:])
        # big xs8 DMA last (split in two so row 0..67 lands first)
        xs8 = cst.tile([96, 2, H + 2, W], fp8, name="xs8", tag="xs8")
        xs_flat = xs8[:].rearrange("p a b c -> p (a b c)")
        nc.sync.dma_start(xs_flat[:, 0:XJ], xs_d[:, 0:XJ])
        nc.sync.dma_start(xs_flat[:, XJ:2 * XJ], xs_d[:, XJ:2 * XJ])

        dwv_res = [res.tile([96, N], bf16, name=f"dwv{i}", tag=f"dwv{i}")
                   for i in range(2)]

        smx = ctx.enter_context(tc.tile_pool(name="smx", bufs=1))

        # ================= PHASE A =================
        n_blk_total = H

        pa_stack = ExitStack()
        stg = pa_stack.enter_context(tc.tile_pool(name="stg", bufs=2))
        tro = pa_stack.enter_context(tc.tile_pool(name="tro", bufs=1))
        cvps = pa_stack.enter_context(
            tc.tile_pool(name="cvps", bufs=1, space="PSUM"))
        scps = pa_stack.enter_context(
            tc.tile_pool(name="scps", bufs=1, space="PSUM"))

        psc = scps.tile([96, 2, 96], f32, name="psc", tag="psc")
        blk_count = [0, 0]

        # ---- PE warm-up: dummy matmuls on the (small, early) weight
        # tiles while the 3.2MB xs8 DMA streams in; keeps HAM at 8/8.
        warm = cvps.tile([128, 4, 128], f32, name="cv0", tag="cv0")
        wflat = wt[0][:].rearrange("p a b c -> p (a b c)")
        wout = warm[0:96, :, :].rearrange("p a b -> p (a b)")
        for _ in range(72):
            nc.tensor.matmul(wout[:, 0:480], wt[0][:, 0, 0, 0:96],
                             wflat[:, 0:480], start=True, stop=True,
                             skip_group_check=True)

        def emit_scores(trts, nrows):
            for hf in range(2):
                for blk in range(nrows):
                    nc.tensor.matmul(
                        psc[:, hf, :],
                        trts[("q", hf)][:, blk, :],
                        trts[("k", hf)][:, blk, :],
                        start=(blk_count[hf] == 0),
                        stop=(blk_count[hf] == n_blk_total - 1),
                        skip_group_check=True)
                    blk_count[hf] += 1

        pending_scores = []
        ecnt = [0]

        for (r0, nw, cw0) in ROTS:
            stgs = {}
            for (p, hf) in (("q", 0), ("q", 1), ("k", 0), ("k", 1)):
                for ci, ncw in ((0, cw0), (1, nw - cw0)):
                    stgs[(p, hf, ci)] = stg.tile(
                        [96, 4 * ncw, 128], bf16,
                        name=f"s{p}{hf}{ci}", tag=f"s{p}{hf}{ci}")
            for g in range(5):
                Mg = GROUP_M[g]
                pst = [cvps.tile([128, 4, 128], f32, name=f"cv{w}",
                                 tag=f"cv{w}") for w in range(nw)]
                for ti, t in enumerate(TAP_ORDER):
                    oy, ox = OFFS[t]
                    (a0, a1), (b0, b1) = xr(ox)
                    for w in range(nw):
                        rw = r0 + 4 * w
                        nc.tensor.matmul(
                            pst[w][0:Mg, :, a0:a1],
                            wt[g][:, t, :, :],
                            xs8[:, :, 1 + rw + oy:5 + rw + oy, b0:b1],
                            start=(ti == 0), stop=(ti == 8),
                            perf_mode=DR,
                            skip_group_check=True)
                # evacuate, splitting psum partitions by segment.
                # HW rule: a partition access starting at base b may span at
                # most 128 (b=0), 64 (b=64), else 32 partitions - split
                # pieces to respect both src and dst bases.
                def _allowed(bp):
                    if bp == 0:
                        return 128
                    if bp % 64 == 0:
                        return 64
                    return 32

                for w in range(nw):
                    rloc = 4 * w          # rows within rotation
                    for (dst_key, plo, phi, olo) in GROUP_SEGS[g]:
                        (p, hf) = dst_key
                        cur = plo
                        while cur < phi:
                            od = olo + (cur - plo)
                            npart = min(phi - cur, _allowed(cur),
                                        _allowed(od))
                            src = pst[w][cur:cur + npart, :, :]
                            if p == "v":
                                seg = (r0 + rloc) * W
                                dst = dwv_res[hf][od:od + npart,
                                                  seg:seg + 512]\
                                    .rearrange("p (r x) -> p r x", x=128)
                            else:
                                ci = 0 if w < cw0 else 1
                                wloc = rloc - (0 if w < cw0 else 4 * cw0)
                                dst = stgs[(p, hf, ci)][od:od + npart,
                                                        wloc:wloc + 4, :]
                            if ecnt[0] % 2 == 0:
                                nc.scalar.copy(dst, src)
                            else:
                                nc.vector.tensor_copy(dst, src)
                            ecnt[0] += 1
                            cur += npart
            # DMA transposes for this rotation's q/k chunks
            for ci, ncw in ((0, cw0), (1, nw - cw0)):
                tr = {}
                for (p, hf) in (("q", 0), ("q", 1), ("k", 0), ("k", 1)):
                    tt = tro.tile([128, 4 * ncw, 96], bf16,
                                  name=f"t{p}{hf}{ci}", tag=f"t{p}{hf}{ci}")
                    nc.sync.dma_start_transpose(
                        tt[:], stgs[(p, hf, ci)][:].rearrange(
                            "p r x -> p (r x)"))
                    tr[(p, hf)] = tt
                pending_scores.append((tr, 4 * ncw))
            while len(pending_scores) > 2:
                emit_scores(*pending_scores.pop(0))
        while pending_scores:
            emit_scores(*pending_scores.pop(0))

        # ================= PHASE B: softmax + attn =================
        ex = []
        rr_ = []
        for hf in range(2):
            scl = smx.tile([96, 96], f32, name=f"scl{hf}", tag=f"scl{hf}")
            nc.vector.tensor_scalar(scl[:], psc[:, hf, :], tsc[:, hf:hf + 1],
                                    None, OP.mult)
            nm = smx.tile([96, 1], f32, name=f"nm{hf}", tag=f"nm{hf}")
            nc.vector.tensor_reduce(nm[:], scl[:], AX.X, OP.max, negate=True)
            e = smx.tile([96, 96], f32, name=f"e{hf}", tag=f"e{hf}")
            nc.scalar.activation(e[:], scl[:], AF.Exp, bias=nm[:, 0:1])
            sm = smx.tile([96, 1], f32, name=f"sm{hf}", tag=f"sm{hf}")
            nc.vector.tensor_reduce(sm[:], e[:], AX.X, OP.add)
            r = smx.tile([96, 1], f32, name=f"r{hf}", tag=f"r{hf}")
            nc.vector.reciprocal(r[:], sm[:])
            ex.append(e)
            rr_.append(r)
        pa_stack.close()

        atstack = ExitStack()
        atps = atstack.enter_context(
            tc.tile_pool(name="atps", bufs=1, space="PSUM"))
        # keep the PE clock warm through the softmax bubble
        warmb = atps.tile([96, 512], f32, name="warmb", tag="warmb")
        for _ in range(20):
            nc.tensor.matmul(warmb[:], ident[:], dwv_res[0][:, 0:512],
                             start=True, stop=True, skip_group_check=True)
        r2n = smx.tile([96, 1], f32, name="r2n", tag="r2n")
        nc.vector.tensor_scalar(r2n[:], rr_[1][:], neglam[0:96, 0:1],
                                None, OP.mult)
        a1 = smx.tile([96, 96], f32, name="a1", tag="a1")
        nc.scalar.mul(a1[:], ex[0][:], rr_[0][:, 0:1])
        attn = smx.tile([96, 96], bf16, name="attn", tag="attn")
        nc.vector.scalar_tensor_tensor(attn[:], ex[1][:], r2n[:, 0:1],
                                       a1[:], OP.mult, OP.add)
        pt = atps.tile([96, 96], bf16, name="pt", tag="pt")
        nc.tensor.transpose(pt[:], attn[:], ident[:])
        attnT = smx.tile([96, 96], bf16, name="attnT", tag="attnT")
        nc.scalar.copy(attnT[:], pt[:])
        atstack.close()

        # ================= PHASE C (5-stage pipeline) =================
        # stats weight w96 = 1/256 full [96,96]: its matmul output is
        # already the partition-broadcast channel-sum, so no separate
        # broadcast matmul / copy is needed.
        with tc.tile_pool(name="yp", bufs=4) as yp, \
             tc.tile_pool(name="op_", bufs=3) as op_, \
             tc.tile_pool(name="yps", bufs=2, space="PSUM") as yps, \
             tc.tile_pool(name="sqps", bufs=2, space="PSUM") as sqps, \
             tc.tile_pool(name="ops", bufs=1, space="PSUM") as ops:
            py2 = {}
            ysb = {}
            yy2 = {}
            pss = {}
            rsb = {}
            ys8 = {}
            po2 = {}
            for it in range(NCH + 4):
                cA = it            # stage A chunk
                cB = it - 1
                cC = it - 2
                cD = it - 3
                cE = it - 4
                if cA < NCH:
                    seg = cA * CH
                    py2[cA] = yps.tile([96, 2, CH], f32, name="y2",
                                       tag="y2")
                    for hf in range(2):
                        nc.tensor.matmul(py2[cA][:, hf, :], attnT[:],
                                         dwv_res[hf][:, seg:seg + CH],
                                         start=True, stop=True,
                                         skip_group_check=True)
                    ysb[cA] = yp.tile([96, 2, CH], bf16, name="ysb",
                                      tag="ysb")
                    nc.scalar.copy(ysb[cA][:], py2[cA][:])
                    py2.pop(cA - 2, None)
                if 0 <= cB < NCH:
                    yy2[cB] = yp.tile([96, 2, CH], bf16, name="yy2",
                                      tag="yy2")
                    nc.gpsimd.tensor_tensor(yy2[cB][:], ysb[cB][:],
                                            ysb[cB][:], OP.mult)
                if 0 <= cC < NCH:
                    pss[cC] = sqps.tile([96, CH], f32, name="ss", tag="ss")
                    # two discarded warm-keeper matmuls keep PE duty high
                    # enough that HAM stays at full clock through phase C
                    nc.tensor.matmul(pss[cC][:], w96[:], yy2[cC][:, 0, :],
                                     start=True, stop=True,
                                     skip_group_check=True)
                    nc.tensor.matmul(pss[cC][:], w96[:], yy2[cC][:, 0, :],
                                     start=True, stop=False,
                                     skip_group_check=True)
                    nc.tensor.matmul(pss[cC][:], w96[:], yy2[cC][:, 1, :],
                                     start=False, stop=True,
                                     skip_group_check=True)
                    rsb[cC] = op_.tile([96, CH], bf16, name="rs", tag="rs")
                    nc.scalar.activation(rsb[cC][:], pss[cC][:],
                                         AF.Abs_reciprocal_sqrt,
                                         bias=epsd[:, 0:1],
                                         scale=1.0 / 192.0)
                    del yy2[cC], pss[cC]
                if 0 <= cD < NCH:
                    ys8[cD] = yp.tile([96, 2, CH], fp8, name="ys8",
                                      tag="ys8")
                    rb3 = rsb[cD][:].rearrange("p (o n) -> p o n", o=1)\
                        .broadcast_to([96, 2, CH])
                    nc.vector.tensor_tensor(ys8[cD][:], ysb[cD][:], rb3,
                                            OP.mult)
                    del rsb[cD], ysb[cD]
                if 0 <= cE < NCH:
                    seg = cE * CH
                    po2[cE] = ops.tile([96, 2, CH], f32, name="po",
                                       tag="po")
                    for mt in range(2):
                        nc.tensor.matmul(po2[cE][:, mt, :], wo8[mt][:],
                                         ys8[cE][:], start=True, stop=True,
                                         perf_mode=DR,
                                         skip_group_check=True)
                    osb = op_.tile([96, 2, CH], f32, name="os", tag="os")
                    nc.vector.tensor_copy(osb[:], po2[cE][:])
                    for mt in range(2):
                        nc.sync.dma_start(
                            out_d[mt * 96:(mt + 1) * 96, seg:seg + CH],
                            osb[:, mt, :])
                    del ys8[cE], po2[cE]
    nc.compile()
    return nc


def _pow2_scale(maxabs, target=128.0):
    if maxabs <= 0:
        return 1.0
    return float(2.0 ** np.floor(np.log2(target / maxabs)))


def _prep_inputs(inputs):
    x = np.asarray(inputs["x"], np.float32)
    norm_w = np.asarray(inputs["norm_w"], np.float32)
    Wq = np.asarray(inputs["Wq"], np.float32)
    Wk = np.asarray(inputs["Wk"], np.float32)
    Wv = np.asarray(inputs["Wv"], np.float32)
    Dq = np.asarray(inputs["Dq"], np.float32)
    Dk = np.asarray(inputs["Dk"], np.float32)
    Dv = np.asarray(inputs["Dv"], np.float32)
    t1 = np.asarray(inputs["t1"], np.float32)
    t2 = np.asarray(inputs["t2"], np.float32)
    hn_w = np.asarray(inputs["hn_w"], np.float32)
    Wo = np.asarray(inputs["Wo"], np.float32)
    lam = float(np.exp(np.sum(inputs["lq1"] * inputs["lk1"],
                              dtype=np.float64))
                - np.exp(np.sum(inputs["lq2"] * inputs["lk2"],
                                dtype=np.float64))
                + LAM_INIT)

    var = x.var(axis=1)
    s = 1.0 / np.sqrt(var + 1e-5)
    xs = (x * s[:, None, :, :]).reshape(B, C, H, W)

    Wf = {"q": Wq * norm_w[None, :], "k": Wk * norm_w[None, :],
          "v": Wv * norm_w[None, :]}
    Dd = {"q": Dq, "k": Dk, "v": Dv}

    in_maps = []
    so_list = []
    for core in range(8):
        b, h = core // 2, core % 2
        sl = slice(h * 192, (h + 1) * 192)
        m = {}
        xpad = np.zeros((96, 2, H + 2, W), np.float32)
        xc = xs[b]
        xpad[:, 0, 1:H + 1, :] = xc[0:96]
        xpad[:, 1, 1:H + 1, :] = xc[96:192]
        m["xs8"] = np.clip(xpad, -224, 224).astype(FP8).reshape(96, -1)

        K3s = {}
        scales = {}
        for nm in ("q", "k", "v"):
            Wh = Wf[nm][sl]
            dh = Dd[nm][sl, 0].reshape(192, 9)
            K3 = Wh[:, :, None] * dh[:, None, :]   # [192 o, 192 cg, 9 t]
            sp = _pow2_scale(np.abs(K3).max())
            scales[nm] = sp
            K3s[nm] = np.clip(K3 * sp, -224, 224)

        # M-group packed conv weights
        tnames = ["q", "q", "k", "k", "v", "v"]
        for g in range(5):
            Mg = GROUP_M[g]
            rows = []
            for mm_ in range(Mg):
                u = 128 * g + mm_
                tname = tnames[u // 96]
                rows.append(K3s[tname][(u % 192)])
            blk = np.stack(rows)                  # [Mg, 192 cg, 9 t]
            w4 = blk.reshape(Mg, 2, 96, 9)        # [m, j, c, t]
            m[f"wg{g}"] = np.ascontiguousarray(
                w4.transpose(2, 3, 1, 0)          # [c, t, j, m]
            ).astype(FP8).reshape(96, -1)

        th = np.array([t1[h, 0, 0], t2[h, 0, 0]], np.float32)
        m["tsc"] = np.broadcast_to(
            (th / (scales["q"] * scales["k"]))[None, :], (96, 2)
        ).astype(np.float32).copy()

        # out-projection (fp8 DR, r folded into ys8 = 16*yhat)
        Wo_hf = Wo[:, sl] * (hn_w[h] * (1.0 - LAM_INIT))[None, :]
        lhsT = Wo_hf.T.astype(np.float32)         # [192 y-ch, 192 out]
        s_o = _pow2_scale(np.abs(lhsT).max() / 16.0)
        lw = np.clip(lhsT * (s_o / 16.0), -448, 448)
        lw = lw.reshape(2, 96, 2, 96)             # [j, c, mt, o]
        for mt in range(2):
            m[f"wo8_{mt}"] = np.ascontiguousarray(
                lw[:, :, mt, :].transpose(1, 0, 2)
            ).astype(FP8).reshape(96, -1)
        so_list.append(s_o)

        sv = scales["v"]
        m["epsd"] = np.full((96, 1), 1e-6 * sv * sv / 256.0, np.float32)
        m["w96"] = np.full((96, 96), 1.0 / 256.0, BF16)
        m["ident"] = np.eye(96, dtype=BF16)
        m["neglam"] = np.full((128, 1), -lam, np.float32)
        in_maps.append(m)
    return in_maps, so_list


def kernel(**inputs):
    from concourse import bass_utils

    if "nc" not in _CACHED:
        _CACHED["nc"] = _build_program()
    nc = _CACHED["nc"]

    in_maps, so_list = _prep_inputs(inputs)
    results = bass_utils.run_bass_kernel_spmd(
        nc, in_maps, core_ids=list(range(8))).results

    x = np.asarray(inputs["x"], np.float32)
    out = np.empty((B, C, N), np.float32)
    for b in range(B):
        out[b] = (results[2 * b]["out"] / so_list[2 * b]
                  + results[2 * b + 1]["out"] / so_list[2 * b + 1])
    out = out.reshape(B, C, H, W) + x
    return out.astype(np.float32)


# revision 27
# speedup vs baseline: 1.6321x; 1.0270x over previous
"""Trainium2 Bass kernel for nn_DTAM (differential transposed-attention module).

Sharding: 8 cores = batch(4) x head(2). Each core computes its (b, h) shard
end-to-end; host does LayerNorm scale precompute, weight folding, and the
final partial-sum + residual merge (including per-core fp8 descale).

v4 design (vs 571us v2 baseline):
- Dense fused 3x3 conv (fp8 DoubleRow) with M=128 OUTPUT PACKING: the 576
  output channels (q/k/v x 2 halves x 96) stream as 5 M-groups
  (4x128 + 1x64) instead of 6x96. The moving-operand port (2B/part/cyc)
  is the hard wall, so fewer streams = directly less PE time.
  Evacuation uses partition-shifted engine copies (PSUM part p -> SBUF
  part q, p != q), verified on HW.
- 4-row conv windows: each matmul streams 512 px (1016-elem moving AP),
  7 PSUM banks rotate per 28 rows; the 8th bank holds the persistent
  score accumulator psc (one accumulation chain per hf over all 128
  row-blocks).
- PE warm-up dummy matmuls run while the 3.2MB xs8 DMA lands, keeping the
  HAM clock-gate at full rate for the first conv rotation.
- Phase C is software-pipelined 5 stages deep across 512-px chunks, so
  the in-chunk PE->ACT->PE->...->DVE dependency chain never stalls PE:
    A: y matmuls (PE), ysb=cast (ACT)
    B: yy=ysb^2 (GPSIMD)
    C: stats matmul (PE), rsqrt (ACT)
    D: broadcast matmul (PE), rbsb copy (ACT), ys8=ysb*r fp8 (DVE)
    E: out-proj fp8-DoubleRow (PE), osb cast (DVE), DMA out
  RMS-norm r is folded into the fp8 proj input (|16*yhat| <= 222 < 448,
  mathematically bounded, no clipping); all scales are pow2-exact, and
  the final pow2 descale happens on host during the partial-sum merge.
"""

import numpy as np
import ml_dtypes
from contextlib import ExitStack

BF16 = ml_dtypes.bfloat16
FP8 = ml_dtypes.float8_e4m3fn

# ---- problem constants (hardcoded per contest rules) ----
B, C, H, W = 4, 192, 128, 128
HEADS = 2
N = H * W                 # 16384
HC = 96
LAM_INIT = 0.8
CH = 512                  # phase C chunk (px)
NCH = N // CH             # 32 chunks

# conv rotations: (start_row, n_windows, windows_in_first_transpose_chunk)
ROTS = [(0, 7, 4), (28, 7, 4), (56, 7, 4), (84, 7, 4), (112, 4, 2)]

# M-group packing of the 576 output channels, order q0,q1,k0,k1,v0,v1.
# Group g covers global channels [128g, 128g+Mg); segment list maps psum
# partition ranges to (tensor, hf, dest channel offset).
GROUP_M = [128, 128, 128, 128, 64]
GROUP_SEGS = [
    [(("q", 0), 0, 96, 0), (("q", 1), 96, 128, 0)],
    [(("q", 1), 0, 64, 32), (("k", 0), 64, 128, 0)],
    [(("k", 0), 0, 32, 64), (("k", 1), 32, 128, 0)],
    [(("v", 0), 0, 96, 0), (("v", 1), 96, 128, 0)],
    [(("v", 1), 0, 64, 32)],
]

_CACHED = {}


def _build_program():
    import concourse.bass as bass
    import concourse.bacc as bacc
    import concourse.tile as tile
    from concourse import mybir

    f32 = mybir.dt.float32
    bf16 = mybir.dt.bfloat16
    fp8 = mybir.dt.float8e4
    AF = mybir.ActivationFunctionType
    OP = mybir.AluOpType
    AX = mybir.AxisListType
    DR = mybir.MatmulPerfMode.DoubleRow

    nc = bacc.Bacc("TRN2", target_bir_lowering=False, debug=False,
                   num_devices=8)

    # ---- DRAM I/O ----
    XJ = (H + 2) * W
    xs_d = nc.dram_tensor("xs8", [96, 2 * XJ], fp8, kind="ExternalInput")
    wg_d = [nc.dram_tensor(f"wg{g}", [96, 9 * 2 * GROUP_M[g]], fp8,
                           kind="ExternalInput") for g in range(5)]
    wo8_d = [nc.dram_tensor(f"wo8_{mt}", [96, 2 * 96], fp8,
                            kind="ExternalInput") for mt in range(2)]
    w96_d = nc.dram_tensor("w96", [96, 96], bf16, kind="ExternalInput")
    ident_d = nc.dram_tensor("ident", [96, 96], bf16, kind="ExternalInput")
    neglam_d = nc.dram_tensor("neglam", [128, 1], f32, kind="ExternalInput")
    tsc_d = nc.dram_tensor("tsc", [96, 2], f32, kind="ExternalInput")
    epsd_d = nc.dram_tensor("epsd", [96, 1], f32, kind="ExternalInput")
    out_d = nc.dram_tensor("out", [192, N], f32, kind="ExternalOutput")

    OFFS = [(t // 3 - 1, t % 3 - 1) for t in range(9)]
    TAP_ORDER = [4] + [t for t in range(9) if t != 4]

    def xr(ox):
        if ox == -1:
            return (1, 128), (0, 127)
        if ox == 1:
            return (0, 127), (1, 128)
        return (0, 128), (0, 128)

    with tile.TileContext(nc) as tc, ExitStack() as ctx:
        cst = ctx.enter_context(tc.tile_pool(name="cst", bufs=1))
        res = ctx.enter_context(tc.tile_pool(name="res", bufs=1))

        # ---- constants: small DMAs first so warm-up can start early ----
        wt = []
        for g in range(5):
            t = cst.tile([96, 9, 2, GROUP_M[g]], fp8, name=f"wg{g}",
                         tag=f"wg{g}")
            nc.sync.dma_start(t[:].rearrange("p a b c -> p (a b c)"),
                              wg_d[g][:])
            wt.append(t)
        wo8 = []
        for mt in range(2):
            t = cst.tile([96, 2, 96], fp8, name=f"wo8{mt}", tag=f"wo8{mt}")
            nc.sync.dma_start(t[:].rearrange("p a b -> p (a b)"),
                              wo8_d[mt][:])
            wo8.append(t)
        w96 = cst.tile([96, 96], bf16, name="w96", tag="w96")
        ident = cst.tile([96, 96], bf16, name="id", tag="id")
        neglam = cst.tile([128, 1], f32, name="nl", tag="nl")
        tsc = cst.tile([96, 2], f32, name="tsc", tag="tsc")
        epsd = cst.tile([96, 1], f32, name="epsd", tag="epsd")
        nc.sync.dma_start(w96[:], w96_d[:])
        nc.sync.dma_start(ident[:], ident_d[:])
        nc.sync.dma_start(neglam[:], neglam_d[:])
        nc.sync.dma_start(tsc[:], tsc_d[:])
        nc.sync.dma_start(epsd[:], epsd_d[:])
        # big xs8 DMA last (split in two so row 0..67 lands first)
        xs8 = cst.tile([96, 2, H + 2, W], fp8, name="xs8", tag="xs8")
        xs_flat = xs8[:].rearrange("p a b c -> p (a b c)")
        nc.sync.dma_start(xs_flat[:, 0:XJ], xs_d[:, 0:XJ])
        nc.sync.dma_start(xs_flat[:, XJ:2 * XJ], xs_d[:, XJ:2 * XJ])

        dwv_res = [res.tile([96, N], bf16, name=f"dwv{i}", tag=f"dwv{i}")
                   for i in range(2)]

        smx = ctx.enter_context(tc.tile_pool(name="smx", bufs=1))

        # ================= PHASE A =================
        n_blk_total = H

        pa_stack = ExitStack()
        stg = pa_stack.enter_context(tc.tile_pool(name="stg", bufs=2))
        tro = pa_stack.enter_context(tc.tile_pool(name="tro", bufs=1))
        cvps = pa_stack.enter_context(
            tc.tile_pool(name="cvps", bufs=1, space="PSUM"))
        scps = pa_stack.enter_context(
            tc.tile_pool(name="scps", bufs=1, space="PSUM"))

        psc = scps.tile([96, 2, 96], f32, name="psc", tag="psc")
        blk_count = [0, 0]

        # ---- PE warm-up: dummy matmuls on the (small, early) weight
        # tiles while the 3.2MB xs8 DMA streams in; keeps HAM at 8/8.
        warm = cvps.tile([128, 4, 128], f32, name="cv0", tag="cv0")
        wflat = wt[0][:].rearrange("p a b c -> p (a b c)")
        wout = warm[0:96, :, :].rearrange("p a b -> p (a b)")
        for _ in range(72):
            nc.tensor.matmul(wout[:, 0:480], wt[0][:, 0, 0, 0:96],
                             wflat[:, 0:480], start=True, stop=True,
                             skip_group_check=True)

        def emit_scores(trts, nrows):
            for hf in range(2):
                for blk in range(nrows):
                    nc.tensor.matmul(
                        psc[:, hf, :],
                        trts[("q", hf)][:, blk, :],
                        trts[("k", hf)][:, blk, :],
                        start=(blk_count[hf] == 0),
                        stop=(blk_count[hf] == n_blk_total - 1),
                        skip_group_check=True)
                    blk_count[hf] += 1

        pending_scores = []
        ecnt = [0]

        for (r0, nw, cw0) in ROTS:
            stgs = {}
            for (p, hf) in (("q", 0), ("q", 1), ("k", 0), ("k", 1)):
                for ci, ncw in ((0, cw0), (1, nw - cw0)):
                    stgs[(p, hf, ci)] = stg.tile(
                        [96, 4 * ncw, 128], bf16,
                        name=f"s{p}{hf}{ci}", tag=f"s{p}{hf}{ci}")
            for g in range(5):
                Mg = GROUP_M[g]
                pst = [cvps.tile([128, 4, 128], f32, name=f"cv{w}",
                                 tag=f"cv{w}") for w in range(nw)]
                for ti, t in enumerate(TAP_ORDER):
                    oy, ox = OFFS[t]
                    (a0, a1), (b0, b1) = xr(ox)
                    for w in range(nw):
                        rw = r0 + 4 * w
                        nc.tensor.matmul(
                            pst[w][0:Mg, :, a0:a1],
                            wt[g][:, t, :, :],
                            xs8[:, :, 1 + rw + oy:5 + rw + oy, b0:b1],
                            start=(ti == 0), stop=(ti == 8),
                            perf_mode=DR,
                            skip_group_check=True)
                # evacuate, splitting psum partitions by segment.
                # HW rule: a partition access starting at base b may span at
                # most 128 (b=0), 64 (b=64), else 32 partitions - split
                # pieces to respect both src and dst bases.
                def _allowed(bp):
                    if bp == 0:
                        return 128
                    if bp % 64 == 0:
                        return 64
                    return 32

                for w in range(nw):
                    rloc = 4 * w          # rows within rotation
                    for (dst_key, plo, phi, olo) in GROUP_SEGS[g]:
                        (p, hf) = dst_key
                        cur = plo
                        while cur < phi:
                            od = olo + (cur - plo)
                            npart = min(phi - cur, _allowed(cur),
                                        _allowed(od))
                            src = pst[w][cur:cur + npart, :, :]
                            if p == "v":
                                seg = (r0 + rloc) * W
                                dst = dwv_res[hf][od:od + npart,
                                                  seg:seg + 512]\
                                    .rearrange("p (r x) -> p r x", x=128)
                            else:
                                ci = 0 if w < cw0 else 1
                                wloc = rloc - (0 if w < cw0 else 4 * cw0)
                                dst = stgs[(p, hf, ci)][od:od + npart,
                                                        wloc:wloc + 4, :]
                            if ecnt[0] % 2 == 0:
                                nc.scalar.copy(dst, src)
                            else:
                                nc.vector.tensor_copy(dst, src)
                            ecnt[0] += 1
                            cur += npart
            # DMA transposes for this rotation's q/k chunks
            for ci, ncw in ((0, cw0), (1, nw - cw0)):
                tr = {}
                for (p, hf) in (("q", 0), ("q", 1), ("k", 0), ("k", 1)):
                    tt = tro.tile([128, 4 * ncw, 96], bf16,
                                  name=f"t{p}{hf}{ci}", tag=f"t{p}{hf}{ci}")
                    nc.sync.dma_start_transpose(
                        tt[:], stgs[(p, hf, ci)][:].rearrange(
                            "p r x -> p (r x)"))
                    tr[(p, hf)] = tt
                pending_scores.append((tr, 4 * ncw))
            while len(pending_scores) > 2:
                emit_scores(*pending_scores.pop(0))
        while pending_scores:
            emit_scores(*pending_scores.pop(0))

        # ================= PHASE B: softmax + attn =================
        ex = []
        rr_ = []
        for hf in range(2):
            scl = smx.tile([96, 96], f32, name=f"scl{hf}", tag=f"scl{hf}")
            nc.vector.tensor_scalar(scl[:], psc[:, hf, :], tsc[:, hf:hf + 1],
                                    None, OP.mult)
            nm = smx.tile([96, 1], f32, name=f"nm{hf}", tag=f"nm{hf}")
            nc.vector.tensor_reduce(nm[:], scl[:], AX.X, OP.max, negate=True)
            e = smx.tile([96, 96], f32, name=f"e{hf}", tag=f"e{hf}")
            nc.scalar.activation(e[:], scl[:], AF.Exp, bias=nm[:, 0:1])
            sm = smx.tile([96, 1], f32, name=f"sm{hf}", tag=f"sm{hf}")
            nc.vector.tensor_reduce(sm[:], e[:], AX.X, OP.add)
            r = smx.tile([96, 1], f32, name=f"r{hf}", tag=f"r{hf}")
            nc.vector.reciprocal(r[:], sm[:])
            ex.append(e)
            rr_.append(r)
        pa_stack.close()

        atstack = ExitStack()
        atps = atstack.enter_context(
            tc.tile_pool(name="atps", bufs=1, space="PSUM"))
        # keep the PE clock warm through the softmax bubble
        warmb = atps.tile([96, 512], f32, name="warmb", tag="warmb")
        for _ in range(20):
            nc.tensor.matmul(warmb[:], ident[:], dwv_res[0][:, 0:512],
                             start=True, stop=True, skip_group_check=True)
        r2n = smx.tile([96, 1], f32, name="r2n", tag="r2n")
        nc.vector.tensor_scalar(r2n[:], rr_[1][:], neglam[0:96, 0:1],
                                None, OP.mult)
        a1 = smx.tile([96, 96], f32, name="a1", tag="a1")
        nc.scalar.mul(a1[:], ex[0][:], rr_[0][:, 0:1])
        attn = smx.tile([96, 96], bf16, name="attn", tag="attn")
        nc.vector.scalar_tensor_tensor(attn[:], ex[1][:], r2n[:, 0:1],
                                       a1[:], OP.mult, OP.add)
        pt = atps.tile([96, 96], bf16, name="pt", tag="pt")
        nc.tensor.transpose(pt[:], attn[:], ident[:])
        attnT = smx.tile([96, 96], bf16, name="attnT", tag="attnT")
        nc.scalar.copy(attnT[:], pt[:])
        atstack.close()

        # ================= PHASE C (5-stage pipeline) =================
        # stats weight w96 = 1/256 full [96,96]: its matmul output is
        # already the partition-broadcast channel-sum, so no separate
        # broadcast matmul / copy is needed.
        with tc.tile_pool(name="yp", bufs=4) as yp, \
             tc.tile_pool(name="op_", bufs=3) as op_, \
             tc.tile_pool(name="yps", bufs=2, space="PSUM") as yps, \
             tc.tile_pool(name="sqps", bufs=2, space="PSUM") as sqps, \
             tc.tile_pool(name="ops", bufs=1, space="PSUM") as ops:
            py2 = {}
            ysb = {}
            yy2 = {}
            pss = {}
            rsb = {}
            ys8 = {}
            po2 = {}
            for it in range(NCH + 4):
                cA = it            # stage A chunk
                cB = it - 1
                cC = it - 2
                cD = it - 3
                cE = it - 4
                if cA < NCH:
                    seg = cA * CH
                    py2[cA] = yps.tile([96, 2, CH], f32, name="y2",
                                       tag="y2")
                    for hf in range(2):
                        nc.tensor.matmul(py2[cA][:, hf, :], attnT[:],
                                         dwv_res[hf][:, seg:seg + CH],
                                         start=True, stop=True,
                                         skip_group_check=True)
                    ysb[cA] = yp.tile([96, 2, CH], bf16, name="ysb",
                                      tag="ysb")
                    nc.scalar.copy(ysb[cA][:], py2[cA][:])
                    py2.pop(cA - 2, None)
                if 0 <= cB < NCH:
                    yy2[cB] = yp.tile([96, 2, CH], bf16, name="yy2",
                                      tag="yy2")
                    nc.gpsimd.tensor_tensor(yy2[cB][:, 0, :],
                                            ysb[cB][:, 0, :],
                                            ysb[cB][:, 0, :], OP.mult)
                    nc.vector.tensor_tensor(yy2[cB][:, 1, :],
                                            ysb[cB][:, 1, :],
                                            ysb[cB][:, 1, :], OP.mult)
                if 0 <= cC < NCH:
                    pss[cC] = sqps.tile([96, CH], f32, name="ss", tag="ss")
                    # two discarded warm-keeper matmuls keep PE duty high
                    # enough that HAM stays at full clock through phase C
                    nc.tensor.matmul(pss[cC][:], w96[:], yy2[cC][:, 0, :],
                                     start=True, stop=True,
                                     skip_group_check=True)
                    nc.tensor.matmul(pss[cC][:], w96[:], yy2[cC][:, 0, :],
                                     start=True, stop=False,
                                     skip_group_check=True)
                    nc.tensor.matmul(pss[cC][:], w96[:], yy2[cC][:, 1, :],
                                     start=False, stop=True,
                                     skip_group_check=True)
                    rsb[cC] = op_.tile([96, CH], bf16, name="rs", tag="rs")
                    nc.scalar.activation(rsb[cC][:], pss[cC][:],
                                         AF.Abs_reciprocal_sqrt,
                                         bias=epsd[:, 0:1],
                                         scale=1.0 / 192.0)
                    del yy2[cC], pss[cC]
                if 0 <= cD < NCH:
                    ys8[cD] = yp.tile([96, 2, CH], fp8, name="ys8",
                                      tag="ys8")
                    rb3 = rsb[cD][:].rearrange("p (o n) -> p o n", o=1)\
                        .broadcast_to([96, 2, CH])
                    nc.vector.tensor_tensor(ys8[cD][:], ysb[cD][:], rb3,
                                            OP.mult)
                    del rsb[cD], ysb[cD]
                if 0 <= cE < NCH:
                    seg = cE * CH
                    po2[cE] = ops.tile([96, 2, CH], f32, name="po",
                                       tag="po")
                    for mt in range(2):
                        nc.tensor.matmul(po2[cE][:, mt, :], wo8[mt][:],
                                         ys8[cE][:], start=True, stop=True,
                                         perf_mode=DR,
                                         skip_group_check=True)
                    osb = op_.tile([96, 2, CH], f32, name="os", tag="os")
                    nc.scalar.copy(osb[:, 0, :], po2[cE][:, 0, :])
                    nc.vector.tensor_copy(osb[:, 1, :], po2[cE][:, 1, :])
                    for mt in range(2):
                        nc.sync.dma_start(
                            out_d[mt * 96:(mt + 1) * 96, seg:seg + CH],
                            osb[:, mt, :])
                    del ys8[cE], po2[cE]
    nc.compile()
    return nc


def _pow2_scale(maxabs, target=128.0):
    if maxabs <= 0:
        return 1.0
    return float(2.0 ** np.floor(np.log2(target / maxabs)))


def _prep_inputs(inputs):
    x = np.asarray(inputs["x"], np.float32)
    norm_w = np.asarray(inputs["norm_w"], np.float32)
    Wq = np.asarray(inputs["Wq"], np.float32)
    Wk = np.asarray(inputs["Wk"], np.float32)
    Wv = np.asarray(inputs["Wv"], np.float32)
    Dq = np.asarray(inputs["Dq"], np.float32)
    Dk = np.asarray(inputs["Dk"], np.float32)
    Dv = np.asarray(inputs["Dv"], np.float32)
    t1 = np.asarray(inputs["t1"], np.float32)
    t2 = np.asarray(inputs["t2"], np.float32)
    hn_w = np.asarray(inputs["hn_w"], np.float32)
    Wo = np.asarray(inputs["Wo"], np.float32)
    lam = float(np.exp(np.sum(inputs["lq1"] * inputs["lk1"],
                              dtype=np.float64))
                - np.exp(np.sum(inputs["lq2"] * inputs["lk2"],
                                dtype=np.float64))
                + LAM_INIT)

    var = x.var(axis=1)
    s = 1.0 / np.sqrt(var + 1e-5)
    xs = (x * s[:, None, :, :]).reshape(B, C, H, W)

    Wf = {"q": Wq * norm_w[None, :], "k": Wk * norm_w[None, :],
          "v": Wv * norm_w[None, :]}
    Dd = {"q": Dq, "k": Dk, "v": Dv}

    in_maps = []
    so_list = []
    for core in range(8):
        b, h = core // 2, core % 2
        sl = slice(h * 192, (h + 1) * 192)
        m = {}
        xpad = np.zeros((96, 2, H + 2, W), np.float32)
        xc = xs[b]
        xpad[:, 0, 1:H + 1, :] = xc[0:96]
        xpad[:, 1, 1:H + 1, :] = xc[96:192]
        m["xs8"] = np.clip(xpad, -224, 224).astype(FP8).reshape(96, -1)

        K3s = {}
        scales = {}
        for nm in ("q", "k", "v"):
            Wh = Wf[nm][sl]
            dh = Dd[nm][sl, 0].reshape(192, 9)
            K3 = Wh[:, :, None] * dh[:, None, :]   # [192 o, 192 cg, 9 t]
            sp = _pow2_scale(np.abs(K3).max())
            scales[nm] = sp
            K3s[nm] = np.clip(K3 * sp, -224, 224)

        # M-group packed conv weights
        tnames = ["q", "q", "k", "k", "v", "v"]
        for g in range(5):
            Mg = GROUP_M[g]
            rows = []
            for mm_ in range(Mg):
                u = 128 * g + mm_
                tname = tnames[u // 96]
                rows.append(K3s[tname][(u % 192)])
            blk = np.stack(rows)                  # [Mg, 192 cg, 9 t]
            w4 = blk.reshape(Mg, 2, 96, 9)        # [m, j, c, t]
            m[f"wg{g}"] = np.ascontiguousarray(
                w4.transpose(2, 3, 1, 0)          # [c, t, j, m]
            ).astype(FP8).reshape(96, -1)

        th = np.array([t1[h, 0, 0], t2[h, 0, 0]], np.float32)
        m["tsc"] = np.broadcast_to(
            (th / (scales["q"] * scales["k"]))[None, :], (96, 2)
        ).astype(np.float32).copy()

        # out-projection (fp8 DR, r folded into ys8 = 16*yhat)
        Wo_hf = Wo[:, sl] * (hn_w[h] * (1.0 - LAM_INIT))[None, :]
        lhsT = Wo_hf.T.astype(np.float32)         # [192 y-ch, 192 out]
        s_o = _pow2_scale(np.abs(lhsT).max() / 16.0)
        lw = np.clip(lhsT * (s_o / 16.0), -448, 448)
        lw = lw.reshape(2, 96, 2, 96)             # [j, c, mt, o]
        for mt in range(2):
            m[f"wo8_{mt}"] = np.ascontiguousarray(
                lw[:, :, mt, :].transpose(1, 0, 2)
            ).astype(FP8).reshape(96, -1)
        so_list.append(s_o)

        sv = scales["v"]
        m["epsd"] = np.full((96, 1), 1e-6 * sv * sv / 256.0, np.float32)
        m["w96"] = np.full((96, 96), 1.0 / 256.0, BF16)
        m["ident"] = np.eye(96, dtype=BF16)
        m["neglam"] = np.full((128, 1), -lam, np.float32)
        in_maps.append(m)
    return in_maps, so_list


def kernel(**inputs):
    from concourse import bass_utils

    if "nc" not in _CACHED:
        _CACHED["nc"] = _build_program()
    nc = _CACHED["nc"]

    in_maps, so_list = _prep_inputs(inputs)
    results = bass_utils.run_bass_kernel_spmd(
        nc, in_maps, core_ids=list(range(8))).results

    x = np.asarray(inputs["x"], np.float32)
    out = np.empty((B, C, N), np.float32)
    for b in range(B):
        out[b] = (results[2 * b]["out"] / so_list[2 * b]
                  + results[2 * b + 1]["out"] / so_list[2 * b + 1])
    out = out.reshape(B, C, H, W) + x
    return out.astype(np.float32)


# revision 30
# speedup vs baseline: 1.8277x; 1.1198x over previous
"""Trainium2 Bass kernel for nn_DTAM (differential transposed-attention module).

Sharding: 8 cores = batch(4) x head(2). Each core computes its (b, h) shard
end-to-end; host does LayerNorm scale precompute, weight folding, and the
final partial-sum + residual merge (including per-core fp8 descale).

v4 design (vs 571us v2 baseline):
- Dense fused 3x3 conv (fp8 DoubleRow) with M=128 OUTPUT PACKING: the 576
  output channels (q/k/v x 2 halves x 96) stream as 5 M-groups
  (4x128 + 1x64) instead of 6x96. The moving-operand port (2B/part/cyc)
  is the hard wall, so fewer streams = directly less PE time.
  Evacuation uses partition-shifted engine copies (PSUM part p -> SBUF
  part q, p != q), verified on HW.
- 4-row conv windows: each matmul streams 512 px (1016-elem moving AP),
  7 PSUM banks rotate per 28 rows; the 8th bank holds the persistent
  score accumulator psc (one accumulation chain per hf over all 128
  row-blocks).
- PE warm-up dummy matmuls run while the 3.2MB xs8 DMA lands, keeping the
  HAM clock-gate at full rate for the first conv rotation.
- Phase C is software-pipelined 5 stages deep across 512-px chunks, so
  the in-chunk PE->ACT->PE->...->DVE dependency chain never stalls PE:
    A: y matmuls (PE), ysb=cast (ACT)
    B: yy=ysb^2 (GPSIMD)
    C: stats matmul (PE), rsqrt (ACT)
    D: broadcast matmul (PE), rbsb copy (ACT), ys8=ysb*r fp8 (DVE)
    E: out-proj fp8-DoubleRow (PE), osb cast (DVE), DMA out
  RMS-norm r is folded into the fp8 proj input (|16*yhat| <= 222 < 448,
  mathematically bounded, no clipping); all scales are pow2-exact, and
  the final pow2 descale happens on host during the partial-sum merge.
"""

import numpy as np
import ml_dtypes
from contextlib import ExitStack

BF16 = ml_dtypes.bfloat16
FP8 = ml_dtypes.float8_e4m3fn

# ---- problem constants (hardcoded per contest rules) ----
B, C, H, W = 4, 192, 128, 128
HEADS = 2
N = H * W                 # 16384
HC = 96
LAM_INIT = 0.8
CH = 512                  # phase C chunk (px)
NCH = N // CH             # 32 chunks

# conv rotations: (start_row, n_windows, windows_in_first_transpose_chunk)
ROTS = [(0, 7, 4), (28, 7, 4), (56, 7, 4), (84, 7, 4), (112, 4, 2)]

# M-group packing of the 576 output channels, order q0,q1,k0,k1,v0,v1.
# Group g covers global channels [128g, 128g+Mg); segment list maps psum
# partition ranges to (tensor, hf, dest channel offset).
GROUP_M = [128, 128, 128, 128, 64]
GROUP_SEGS = [
    [(("q", 0), 0, 96, 0), (("q", 1), 96, 128, 0)],
    [(("q", 1), 0, 64, 32), (("k", 0), 64, 128, 0)],
    [(("k", 0), 0, 32, 64), (("k", 1), 32, 128, 0)],
    [(("v", 0), 0, 96, 0), (("v", 1), 96, 128, 0)],
    [(("v", 1), 0, 64, 32)],
]

_CACHED = {}


def _build_program():
    import concourse.bass as bass
    import concourse.bacc as bacc
    import concourse.tile as tile
    from concourse import mybir

    f32 = mybir.dt.float32
    bf16 = mybir.dt.bfloat16
    fp8 = mybir.dt.float8e4
    AF = mybir.ActivationFunctionType
    OP = mybir.AluOpType
    AX = mybir.AxisListType
    DR = mybir.MatmulPerfMode.DoubleRow

    nc = bacc.Bacc("TRN2", target_bir_lowering=False, debug=False,
                   num_devices=8)

    # ---- DRAM I/O ----
    XJ = (H + 2) * W
    xs_d = nc.dram_tensor("xs8", [96, 2 * XJ], fp8, kind="ExternalInput")
    wg_d = [nc.dram_tensor(f"wg{g}", [96, 9 * 2 * GROUP_M[g]], fp8,
                           kind="ExternalInput") for g in range(5)]
    wo8_d = [nc.dram_tensor(f"wo8_{mt}", [96, 2 * 96], fp8,
                            kind="ExternalInput") for mt in range(2)]
    w96_d = nc.dram_tensor("w96", [96, 96], bf16, kind="ExternalInput")
    ident_d = nc.dram_tensor("ident", [96, 96], bf16, kind="ExternalInput")
    neglam_d = nc.dram_tensor("neglam", [128, 1], f32, kind="ExternalInput")
    tsc_d = nc.dram_tensor("tsc", [96, 2], f32, kind="ExternalInput")
    epsd_d = nc.dram_tensor("epsd", [96, 1], f32, kind="ExternalInput")
    out_d = nc.dram_tensor("out", [192, N], f32, kind="ExternalOutput")

    OFFS = [(t // 3 - 1, t % 3 - 1) for t in range(9)]
    TAP_ORDER = [4] + [t for t in range(9) if t != 4]

    def xr(ox):
        if ox == -1:
            return (1, 128), (0, 127)
        if ox == 1:
            return (0, 127), (1, 128)
        return (0, 128), (0, 128)

    with tile.TileContext(nc) as tc, ExitStack() as ctx:
        cst = ctx.enter_context(tc.tile_pool(name="cst", bufs=1))
        res = ctx.enter_context(tc.tile_pool(name="res", bufs=1))

        # ---- constants: small DMAs first so warm-up can start early ----
        wt = []
        for g in range(5):
            t = cst.tile([96, 9, 2, GROUP_M[g]], fp8, name=f"wg{g}",
                         tag=f"wg{g}")
            nc.sync.dma_start(t[:].rearrange("p a b c -> p (a b c)"),
                              wg_d[g][:])
            wt.append(t)
        wo8 = []
        for mt in range(2):
            t = cst.tile([96, 2, 96], fp8, name=f"wo8{mt}", tag=f"wo8{mt}")
            nc.sync.dma_start(t[:].rearrange("p a b -> p (a b)"),
                              wo8_d[mt][:])
            wo8.append(t)
        w96 = cst.tile([96, 96], bf16, name="w96", tag="w96")
        ident = cst.tile([96, 96], bf16, name="id", tag="id")
        neglam = cst.tile([128, 1], f32, name="nl", tag="nl")
        tsc = cst.tile([96, 2], f32, name="tsc", tag="tsc")
        epsd = cst.tile([96, 1], f32, name="epsd", tag="epsd")
        nc.sync.dma_start(w96[:], w96_d[:])
        nc.sync.dma_start(ident[:], ident_d[:])
        nc.sync.dma_start(neglam[:], neglam_d[:])
        nc.sync.dma_start(tsc[:], tsc_d[:])
        nc.sync.dma_start(epsd[:], epsd_d[:])
        # big xs8 DMA last (split in two so row 0..67 lands first)
        xs8 = cst.tile([96, 2, H + 2, W], fp8, name="xs8", tag="xs8")
        xs_flat = xs8[:].rearrange("p a b c -> p (a b c)")
        nc.sync.dma_start(xs_flat[:, 0:XJ], xs_d[:, 0:XJ])
        nc.sync.dma_start(xs_flat[:, XJ:2 * XJ], xs_d[:, XJ:2 * XJ])

        dwv_res = [res.tile([96, N], bf16, name=f"dwv{i}", tag=f"dwv{i}")
                   for i in range(2)]

        smx = ctx.enter_context(tc.tile_pool(name="smx", bufs=1))

        # ================= PHASE A =================
        n_blk_total = H

        pa_stack = ExitStack()
        stg = pa_stack.enter_context(tc.tile_pool(name="stg", bufs=2))
        tro = pa_stack.enter_context(tc.tile_pool(name="tro", bufs=1))
        cvps = pa_stack.enter_context(
            tc.tile_pool(name="cvps", bufs=1, space="PSUM"))
        scps = pa_stack.enter_context(
            tc.tile_pool(name="scps", bufs=1, space="PSUM"))

        psc = scps.tile([96, 2, 96], f32, name="psc", tag="psc")
        blk_count = [0, 0]

        # ---- PE warm-up: dummy matmuls on the (small, early) weight
        # tiles while the 3.2MB xs8 DMA streams in; keeps HAM at 8/8.
        warm = cvps.tile([128, 4, 128], f32, name="cv0", tag="cv0")
        wflat = wt[0][:].rearrange("p a b c -> p (a b c)")
        wout = warm[0:96, :, :].rearrange("p a b -> p (a b)")
        for _ in range(72):
            nc.tensor.matmul(wout[:, 0:480], wt[0][:, 0, 0, 0:96],
                             wflat[:, 0:480], start=True, stop=True,
                             skip_group_check=True)

        def emit_scores(trts, nrows):
            for hf in range(2):
                for blk in range(nrows):
                    nc.tensor.matmul(
                        psc[:, hf, :],
                        trts[("q", hf)][:, blk, :],
                        trts[("k", hf)][:, blk, :],
                        start=(blk_count[hf] == 0),
                        stop=(blk_count[hf] == n_blk_total - 1),
                        skip_group_check=True)
                    blk_count[hf] += 1

        pending_scores = []
        ecnt = [0]

        for (r0, nw, cw0) in ROTS:
            stgs = {}
            for (p, hf) in (("q", 0), ("q", 1), ("k", 0), ("k", 1)):
                for ci, ncw in ((0, cw0), (1, nw - cw0)):
                    stgs[(p, hf, ci)] = stg.tile(
                        [96, 4 * ncw, 128], bf16,
                        name=f"s{p}{hf}{ci}", tag=f"s{p}{hf}{ci}")
            for g in range(3):
                Mg = GROUP_M[g]
                pst = [cvps.tile([128, 4, 128], f32, name=f"cv{w}",
                                 tag=f"cv{w}") for w in range(nw)]
                for ti, t in enumerate(TAP_ORDER):
                    oy, ox = OFFS[t]
                    (a0, a1), (b0, b1) = xr(ox)
                    for w in range(nw):
                        rw = r0 + 4 * w
                        nc.tensor.matmul(
                            pst[w][0:Mg, :, a0:a1],
                            wt[g][:, t, :, :],
                            xs8[:, :, 1 + rw + oy:5 + rw + oy, b0:b1],
                            start=(ti == 0), stop=(ti == 8),
                            perf_mode=DR,
                            skip_group_check=True)
                # evacuate, splitting psum partitions by segment.
                # HW rule: a partition access starting at base b may span at
                # most 128 (b=0), 64 (b=64), else 32 partitions - split
                # pieces to respect both src and dst bases.
                def _allowed(bp):
                    if bp == 0:
                        return 128
                    if bp % 64 == 0:
                        return 64
                    return 32

                for w in range(nw):
                    rloc = 4 * w          # rows within rotation
                    for (dst_key, plo, phi, olo) in GROUP_SEGS[g]:
                        (p, hf) = dst_key
                        cur = plo
                        while cur < phi:
                            od = olo + (cur - plo)
                            npart = min(phi - cur, _allowed(cur),
                                        _allowed(od))
                            src = pst[w][cur:cur + npart, :, :]
                            if p == "v":
                                seg = (r0 + rloc) * W
                                dst = dwv_res[hf][od:od + npart,
                                                  seg:seg + 512]\
                                    .rearrange("p (r x) -> p r x", x=128)
                            else:
                                ci = 0 if w < cw0 else 1
                                wloc = rloc - (0 if w < cw0 else 4 * cw0)
                                dst = stgs[(p, hf, ci)][od:od + npart,
                                                        wloc:wloc + 4, :]
                            if ecnt[0] % 2 == 0:
                                nc.scalar.copy(dst, src)
                            else:
                                nc.vector.tensor_copy(dst, src)
                            ecnt[0] += 1
                            cur += npart
            # DMA transposes for this rotation's q/k chunks
            for ci, ncw in ((0, cw0), (1, nw - cw0)):
                tr = {}
                for (p, hf) in (("q", 0), ("q", 1), ("k", 0), ("k", 1)):
                    tt = tro.tile([128, 4 * ncw, 96], bf16,
                                  name=f"t{p}{hf}{ci}", tag=f"t{p}{hf}{ci}")
                    nc.sync.dma_start_transpose(
                        tt[:], stgs[(p, hf, ci)][:].rearrange(
                            "p r x -> p (r x)"))
                    tr[(p, hf)] = tt
                pending_scores.append((tr, 4 * ncw))
            while len(pending_scores) > 2:
                emit_scores(*pending_scores.pop(0))
        while pending_scores:
            emit_scores(*pending_scores.pop(0))

        # ================= PHASE B: softmax + attn =================
        ex = []
        rr_ = []
        for hf in range(2):
            scl = smx.tile([96, 96], f32, name=f"scl{hf}", tag=f"scl{hf}")
            nc.vector.tensor_scalar(scl[:], psc[:, hf, :], tsc[:, hf:hf + 1],
                                    None, OP.mult)
            nm = smx.tile([96, 1], f32, name=f"nm{hf}", tag=f"nm{hf}")
            nc.vector.tensor_reduce(nm[:], scl[:], AX.X, OP.max, negate=True)
            e = smx.tile([96, 96], f32, name=f"e{hf}", tag=f"e{hf}")
            nc.scalar.activation(e[:], scl[:], AF.Exp, bias=nm[:, 0:1])
            sm = smx.tile([96, 1], f32, name=f"sm{hf}", tag=f"sm{hf}")
            nc.vector.tensor_reduce(sm[:], e[:], AX.X, OP.add)
            r = smx.tile([96, 1], f32, name=f"r{hf}", tag=f"r{hf}")
            nc.vector.reciprocal(r[:], sm[:])
            ex.append(e)
            rr_.append(r)
        pa_stack.close()

        atstack = ExitStack()
        atps = atstack.enter_context(
            tc.tile_pool(name="atps", bufs=1, space="PSUM"))
        # keep the PE clock warm through the softmax bubble
        warmb = atps.tile([96, 512], f32, name="warmb", tag="warmb")
        for _ in range(30):
            nc.tensor.matmul(warmb[:, 0:96], ident[:], ident[:],
                             start=True, stop=True, skip_group_check=True)
        r2n = smx.tile([96, 1], f32, name="r2n", tag="r2n")
        nc.vector.tensor_scalar(r2n[:], rr_[1][:], neglam[0:96, 0:1],
                                None, OP.mult)
        a1 = smx.tile([96, 96], f32, name="a1", tag="a1")
        nc.scalar.mul(a1[:], ex[0][:], rr_[0][:, 0:1])
        attn = smx.tile([96, 96], bf16, name="attn", tag="attn")
        nc.vector.scalar_tensor_tensor(attn[:], ex[1][:], r2n[:, 0:1],
                                       a1[:], OP.mult, OP.add)
        pt = atps.tile([96, 96], bf16, name="pt", tag="pt")
        nc.tensor.transpose(pt[:], attn[:], ident[:])
        attnT = smx.tile([96, 96], bf16, name="attnT", tag="attnT")
        nc.scalar.copy(attnT[:], pt[:])
        atstack.close()

        # ===== PHASE A2+C: v-conv (groups 3,4) interleaved with the =====
        # ===== 5-stage phase C pipeline, 12-row rotations           =====
        # Phase C engine work hides under v-conv PE streams and the PE
        # never idles long enough to re-throttle.
        # stats weight w96 = 1/256 full [96,96]: its matmul output is
        # already the partition-broadcast channel-sum, so no separate
        # broadcast matmul / copy is needed.
        ROTS2 = [(r, 3) for r in range(0, 120, 12)] + [(120, 2)]

        def _allowed(bp):
            if bp == 0:
                return 128
            if bp % 64 == 0:
                return 64
            return 32

        with tc.tile_pool(name="yp", bufs=4) as yp, \
             tc.tile_pool(name="op_", bufs=3) as op_, \
             tc.tile_pool(name="cvp2", bufs=1, space="PSUM") as cvp2, \
             tc.tile_pool(name="yps", bufs=1, space="PSUM") as yps, \
             tc.tile_pool(name="sqps", bufs=1, space="PSUM") as sqps, \
             tc.tile_pool(name="ops", bufs=1, space="PSUM") as ops:
            py2 = {}
            ysb = {}
            yy2 = {}
            pss = {}
            rsb = {}
            ys8 = {}
            po2 = {}

            def phc_iter(it):
                cA = it            # stage A chunk
                cB = it - 1
                cC = it - 2
                cD = it - 3
                cE = it - 4
                if cA < NCH:
                    seg = cA * CH
                    py2[cA] = yps.tile([96, 2, CH], f32, name="y2",
                                       tag="y2")
                    for hf in range(2):
                        nc.tensor.matmul(py2[cA][:, hf, :], attnT[:],
                                         dwv_res[hf][:, seg:seg + CH],
                                         start=True, stop=True,
                                         skip_group_check=True)
                    ysb[cA] = yp.tile([96, 2, CH], bf16, name="ysb",
                                      tag="ysb")
                    nc.scalar.copy(ysb[cA][:], py2[cA][:])
                    py2.pop(cA - 2, None)
                if 0 <= cB < NCH:
                    yy2[cB] = yp.tile([96, 2, CH], bf16, name="yy2",
                                      tag="yy2")
                    nc.gpsimd.tensor_tensor(yy2[cB][:, 0, :],
                                            ysb[cB][:, 0, :],
                                            ysb[cB][:, 0, :], OP.mult)
                    nc.vector.tensor_tensor(yy2[cB][:, 1, :],
                                            ysb[cB][:, 1, :],
                                            ysb[cB][:, 1, :], OP.mult)
                if 0 <= cC < NCH:
                    pss[cC] = sqps.tile([96, CH], f32, name="ss", tag="ss")
                    nc.tensor.matmul(pss[cC][:], w96[:], yy2[cC][:, 0, :],
                                     start=True, stop=False,
                                     skip_group_check=True)
                    nc.tensor.matmul(pss[cC][:], w96[:], yy2[cC][:, 1, :],
                                     start=False, stop=True,
                                     skip_group_check=True)
                    rsb[cC] = op_.tile([96, CH], bf16, name="rs", tag="rs")
                    nc.scalar.activation(rsb[cC][:], pss[cC][:],
                                         AF.Abs_reciprocal_sqrt,
                                         bias=epsd[:, 0:1],
                                         scale=1.0 / 192.0)
                    del yy2[cC], pss[cC]
                if 0 <= cD < NCH:
                    ys8[cD] = yp.tile([96, 2, CH], fp8, name="ys8",
                                      tag="ys8")
                    rb3 = rsb[cD][:].rearrange("p (o n) -> p o n", o=1)\
                        .broadcast_to([96, 2, CH])
                    nc.vector.tensor_tensor(ys8[cD][:], ysb[cD][:], rb3,
                                            OP.mult)
                    del rsb[cD], ysb[cD]
                if 0 <= cE < NCH:
                    seg = cE * CH
                    po2[cE] = ops.tile([96, 2, CH], f32, name="po",
                                       tag="po")
                    for mt in range(2):
                        nc.tensor.matmul(po2[cE][:, mt, :], wo8[mt][:],
                                         ys8[cE][:], start=True, stop=True,
                                         perf_mode=DR,
                                         skip_group_check=True)
                    osb = op_.tile([96, 2, CH], f32, name="os", tag="os")
                    nc.scalar.copy(osb[:, 0, :], po2[cE][:, 0, :])
                    nc.vector.tensor_copy(osb[:, 1, :], po2[cE][:, 1, :])
                    for mt in range(2):
                        nc.sync.dma_start(
                            out_d[mt * 96:(mt + 1) * 96, seg:seg + CH],
                            osb[:, mt, :])
                    del ys8[cE], po2[cE]

            for (r0, nw) in ROTS2:
                for g in (3, 4):
                    Mg = GROUP_M[g]
                    pst = [cvp2.tile([128, 4, 128], f32, name=f"d{w}",
                                     tag=f"d{w}") for w in range(nw)]
                    for ti, t in enumerate(TAP_ORDER):
                        oy, ox = OFFS[t]
                        (a0, a1), (b0, b1) = xr(ox)
                        for w in range(nw):
                            rw = r0 + 4 * w
                            nc.tensor.matmul(
                                pst[w][0:Mg, :, a0:a1],
                                wt[g][:, t, :, :],
                                xs8[:, :, 1 + rw + oy:5 + rw + oy, b0:b1],
                                start=(ti == 0), stop=(ti == 8),
                                perf_mode=DR,
                                skip_group_check=True)
                    for w in range(nw):
                        for (dst_key, plo, phi, olo) in GROUP_SEGS[g]:
                            (p, hf) = dst_key
                            cur = plo
                            while cur < phi:
                                od = olo + (cur - plo)
                                npart = min(phi - cur, _allowed(cur),
                                            _allowed(od))
                                src = pst[w][cur:cur + npart, :, :]
                                seg = (r0 + 4 * w) * W
                                dst = dwv_res[hf][od:od + npart,
                                                  seg:seg + 512]\
                                    .rearrange("p (r x) -> p r x", x=128)
                                if ecnt[0] % 2 == 0:
                                    nc.scalar.copy(dst, src)
                                else:
                                    nc.vector.tensor_copy(dst, src)
                                ecnt[0] += 1
                                cur += npart
                for w in range(nw):
                    phc_iter(r0 // 4 + w)
            for it in range(NCH, NCH + 4):
                phc_iter(it)
    nc.compile()
    return nc


def _pow2_scale(maxabs, target=128.0):
    if maxabs <= 0:
        return 1.0
    return float(2.0 ** np.floor(np.log2(target / maxabs)))


def _prep_inputs(inputs):
    x = np.asarray(inputs["x"], np.float32)
    norm_w = np.asarray(inputs["norm_w"], np.float32)
    Wq = np.asarray(inputs["Wq"], np.float32)
    Wk = np.asarray(inputs["Wk"], np.float32)
    Wv = np.asarray(inputs["Wv"], np.float32)
    Dq = np.asarray(inputs["Dq"], np.float32)
    Dk = np.asarray(inputs["Dk"], np.float32)
    Dv = np.asarray(inputs["Dv"], np.float32)
    t1 = np.asarray(inputs["t1"], np.float32)
    t2 = np.asarray(inputs["t2"], np.float32)
    hn_w = np.asarray(inputs["hn_w"], np.float32)
    Wo = np.asarray(inputs["Wo"], np.float32)
    lam = float(np.exp(np.sum(inputs["lq1"] * inputs["lk1"],
                              dtype=np.float64))
                - np.exp(np.sum(inputs["lq2"] * inputs["lk2"],
                                dtype=np.float64))
                + LAM_INIT)

    var = x.var(axis=1)
    s = 1.0 / np.sqrt(var + 1e-5)
    xs = (x * s[:, None, :, :]).reshape(B, C, H, W)

    Wf = {"q": Wq * norm_w[None, :], "k": Wk * norm_w[None, :],
          "v": Wv * norm_w[None, :]}
    Dd = {"q": Dq, "k": Dk, "v": Dv}

    in_maps = []
    so_list = []
    for core in range(8):
        b, h = core // 2, core % 2
        sl = slice(h * 192, (h + 1) * 192)
        m = {}
        xpad = np.zeros((96, 2, H + 2, W), np.float32)
        xc = xs[b]
        xpad[:, 0, 1:H + 1, :] = xc[0:96]
        xpad[:, 1, 1:H + 1, :] = xc[96:192]
        m["xs8"] = np.clip(xpad, -224, 224).astype(FP8).reshape(96, -1)

        K3s = {}
        scales = {}
        for nm in ("q", "k", "v"):
            Wh = Wf[nm][sl]
            dh = Dd[nm][sl, 0].reshape(192, 9)
            K3 = Wh[:, :, None] * dh[:, None, :]   # [192 o, 192 cg, 9 t]
            sp = _pow2_scale(np.abs(K3).max())
            scales[nm] = sp
            K3s[nm] = np.clip(K3 * sp, -224, 224)

        # M-group packed conv weights
        tnames = ["q", "q", "k", "k", "v", "v"]
        for g in range(5):
            Mg = GROUP_M[g]
            rows = []
            for mm_ in range(Mg):
                u = 128 * g + mm_
                tname = tnames[u // 96]
                rows.append(K3s[tname][(u % 192)])
            blk = np.stack(rows)                  # [Mg, 192 cg, 9 t]
            w4 = blk.reshape(Mg, 2, 96, 9)        # [m, j, c, t]
            m[f"wg{g}"] = np.ascontiguousarray(
                w4.transpose(2, 3, 1, 0)          # [c, t, j, m]
            ).astype(FP8).reshape(96, -1)

        th = np.array([t1[h, 0, 0], t2[h, 0, 0]], np.float32)
        m["tsc"] = np.broadcast_to(
            (th / (scales["q"] * scales["k"]))[None, :], (96, 2)
        ).astype(np.float32).copy()

        # out-projection (fp8 DR, r folded into ys8 = 16*yhat)
        Wo_hf = Wo[:, sl] * (hn_w[h] * (1.0 - LAM_INIT))[None, :]
        lhsT = Wo_hf.T.astype(np.float32)         # [192 y-ch, 192 out]
        s_o = _pow2_scale(np.abs(lhsT).max() / 16.0)
        lw = np.clip(lhsT * (s_o / 16.0), -448, 448)
        lw = lw.reshape(2, 96, 2, 96)             # [j, c, mt, o]
        for mt in range(2):
            m[f"wo8_{mt}"] = np.ascontiguousarray(
                lw[:, :, mt, :].transpose(1, 0, 2)
            ).astype(FP8).reshape(96, -1)
        so_list.append(s_o)

        sv = scales["v"]
        m["epsd"] = np.full((96, 1), 1e-6 * sv * sv / 256.0, np.float32)
        m["w96"] = np.full((96, 96), 1.0 / 256.0, BF16)
        m["ident"] = np.eye(96, dtype=BF16)
        m["neglam"] = np.full((128, 1), -lam, np.float32)
        in_maps.append(m)
    return in_maps, so_list


def kernel(**inputs):
    from concourse import bass_utils

    if "nc" not in _CACHED:
        _CACHED["nc"] = _build_program()
    nc = _CACHED["nc"]

    in_maps, so_list = _prep_inputs(inputs)
    results = bass_utils.run_bass_kernel_spmd(
        nc, in_maps, core_ids=list(range(8))).results

    x = np.asarray(inputs["x"], np.float32)
    out = np.empty((B, C, N), np.float32)
    for b in range(B):
        out[b] = (results[2 * b]["out"] / so_list[2 * b]
                  + results[2 * b + 1]["out"] / so_list[2 * b + 1])
    out = out.reshape(B, C, H, W) + x
    return out.astype(np.float32)
